# revision 1
# baseline (speedup 1.0000x reference)
"""3-layer GCN (GCNConv x3 + LeakyReLU, PyG semantics) on 8 Trainium2 cores.

Strategy (graph-parallel over destination nodes):
  - Nodes are partitioned into 8 contiguous ranges; core c owns range c and
    computes the output rows for its own nodes.
  - Per layer: G = dinv * (H @ W) computed for own nodes (dense phase), then
    AllGather of G so each core holds the full node-feature table in its
    DRAM, then dma_gather over dst-sorted edges + one-hot matmul segment-sum
    into PSUM per 128-dst tile, then epilogue
        H' = lrelu(dinv * gathered_sum + dinv^2 * (H@W) + bias)
    (the dinv^2 term is the self-loop; symmetric normalization
    dinv[s]*dinv[d] is factored into table pre-scale + per-dst post-scale,
    so no per-edge scaling is needed).
  - dma_gather indices are int16, so the gather table (100352 rows) is
    addressed through 4 sub-table views of 25088 rows; edges are bucketed
    by source range.

The Bass program is SPMD: one program, per-core input data. Per
(tile, bucket) section lengths are shared across cores (max over cores,
padded with dummy edges whose one-hot column is zero: dstrel = -1).
Sections are packed back-to-back inside each (group, bucket) span; a
128-edge chunk that straddles a tile boundary is matmul'd once per
overlapped tile with complementary masked one-hots.
"""
import sys

sys.path.insert(0, "/opt/trn_rl_repo")

import numpy as np

import concourse.bacc as bacc
import concourse.mybir as mybir
import concourse.tile as tile
from concourse import library_config
from concourse.bass_utils import run_bass_kernel_spmd
from concourse.masks import make_identity

_F32 = mybir.dt.float32
_BF16 = mybir.dt.bfloat16
_I16 = mybir.dt.int16
P = 128
D = 64
NEG_SLOPE = 0.01


class Cfg:
    def __init__(self, n_nodes=100000, cores=8, group=3, layers=3):
        self.N = n_nodes
        self.CORES = cores
        self.NPC = self.N // cores            # nodes owned per core
        self.TILES = (self.NPC + P - 1) // P  # dst tiles per core
        self.RPC = self.TILES * P             # padded rows per core
        self.GR = cores * self.RPC            # gather-table rows
        self.NBUCK = max(1, -(-self.GR // 25088))
        assert self.GR % self.NBUCK == 0
        self.BUCK_ROWS = self.GR // self.NBUCK
        assert self.BUCK_ROWS <= 32767 or self.GR <= 32767
        self.GROUP = group                    # dst tiles per gather group
        self.LAYERS = layers
        self.ABLATE = set()
        self.GCAP = 8                         # max 128-blocks per dma_gather
        self.SPACK = False                    # single_packet (64-desc cap)
        self.MSGBUFS = 2
        self.QBUFS = 4
        self.PSABUFS = 2
        self.WKBUFS = 3


DEFAULT_CFG = Cfg()


def _preprocess(edge_index, cfg):
    """Sort/bucket/pack edges; build per-core device arrays and metadata."""
    src = np.asarray(edge_index[0], dtype=np.int64)
    dst = np.asarray(edge_index[1], dtype=np.int64)
    N, CORES, NPC, RPC = cfg.N, cfg.CORES, cfg.NPC, cfg.RPC
    TILES, NBUCK, BUCK_ROWS = cfg.TILES, cfg.NBUCK, cfg.BUCK_ROWS

    deg = np.bincount(dst, minlength=N).astype(np.float32) + 1.0  # + self loop
    dinv = (1.0 / np.sqrt(deg)).astype(np.float32)

    gidx = (src // NPC) * RPC + (src % NPC)   # gather-table row of source
    bucket = gidx // BUCK_ROWS
    lidx = gidx % BUCK_ROWS

    owner = dst // NPC
    dloc = dst - owner * NPC
    tile_id = dloc // P
    dstrel = dloc % P

    counts = np.zeros((CORES, TILES, NBUCK), dtype=np.int64)
    np.add.at(counts, (owner, tile_id, bucket), 1)
    order = np.lexsort((bucket, tile_id, owner))
    sl = lidx[order]
    sr = dstrel[order]

    sec_len = counts.max(axis=0)              # [TILES, NBUCK] shared sections
    groups = [list(range(g, min(g + cfg.GROUP, TILES)))
              for g in range(0, TILES, cfg.GROUP)]

    # layout: group -> bucket -> tile sections back-to-back, span padded to
    # a multiple of 128 at the end
    ginfos = []
    tot_blocks = 0
    tot_cols = 0
    for grp in groups:
        gi = {"tiles": list(grp), "blk0": tot_blocks, "col0": tot_cols,
              "spans": {}, "tb": {}}
        gblk = 0
        gcol = 0
        for b in range(NBUCK):
            span_len = int(sec_len[grp, b].sum())
            if span_len == 0:
                continue
            kgb = -(-span_len // P)
            gi["spans"][b] = (gblk, kgb)
            so = 0
            for t in grp:
                stb = int(sec_len[t, b])
                if stb == 0:
                    continue
                j0 = so // P
                njt = (so + stb - 1) // P - j0 + 1
                gi["tb"][(t, b)] = (gblk + j0, njt, gcol, so)
                gcol += njt
                so += stb
            gblk += kgb
        gi["blocks"] = gblk
        gi["ncols"] = gcol
        tot_blocks += gblk
        tot_cols += gcol
        ginfos.append(gi)
    tot_idx = tot_blocks * P

    # boundaries of each core's (t, b) run in the sorted edge list
    cum = np.zeros((CORES, TILES, NBUCK + 1), dtype=np.int64)
    cum[:, :, 1:] = np.cumsum(counts, axis=2)
    flat_counts = counts.sum(axis=2)
    run = np.cumsum(flat_counts.reshape(-1))
    base = np.zeros(CORES * TILES, dtype=np.int64)
    base[1:] = run[:-1]
    base = base.reshape(CORES, TILES)

    per_core = []
    for c in range(CORES):
        lidx_flat = np.zeros(tot_idx, dtype=np.int16)
        drel = np.full((P, tot_cols), -1.0, dtype=np.float32)
        for gi in ginfos:
            for b, (bo, kgb) in gi["spans"].items():
                span_i0 = (gi["blk0"] + bo) * P
                for t in gi["tiles"]:
                    if (t, b) not in gi["tb"]:
                        continue
                    jb, njt, colg, so = gi["tb"][(t, b)]
                    n = int(counts[c, t, b])
                    if n == 0:
                        continue
                    s0 = int(base[c, t] + cum[c, t, b])
                    pos0 = span_i0 + so
                    lidx_flat[pos0:pos0 + n] = sl[s0:s0 + n].astype(np.int16)
                    q = so + np.arange(n)
                    jrel = q // P - so // P
                    pp = (pos0 + np.arange(n)) % P
                    cols = gi["col0"] + colg + jrel
                    drel[pp, cols] = sr[s0:s0 + n].astype(np.float32)
        idx16 = np.tile(lidx_flat.reshape(tot_idx // 16, 16).T, (8, 1)).copy()
        per_core.append({"idx16": idx16, "dstrel": drel})

    meta = {
        "sec_len": sec_len,
        "groups": groups,
        "ginfos": ginfos,
        "tot_idx": tot_idx,
        "tot_cols": tot_cols,
        "dinv": dinv,
    }
    return meta, per_core


def _build_program(meta, cfg):
    ginfos = meta["ginfos"]
    tot_idx = meta["tot_idx"]
    tot_cols = meta["tot_cols"]
    CORES, TILES, RPC, GR = cfg.CORES, cfg.TILES, cfg.RPC, cfg.GR
    NBUCK, BUCK_ROWS = cfg.NBUCK, cfg.BUCK_ROWS

    gblk_max = max(gi["blocks"] for gi in ginfos)
    gcol_max = max(gi["ncols"] for gi in ginfos)
    kmaxb = max((tb[1] for gi in ginfos for tb in gi["tb"].values()),
                default=1)

    nc = bacc.Bacc("TRN2", debug=False)
    nc.num_devices = CORES

    xT_in = nc.dram_tensor("xT", [D, RPC], _F32, kind="ExternalInput")
    dinv1_in = nc.dram_tensor("dinv1", [P, TILES], _F32, kind="ExternalInput")
    dinv2_in = nc.dram_tensor("dinv2", [P, TILES], _F32, kind="ExternalInput")
    w_in = [nc.dram_tensor(f"W{i + 1}", [D, D], _F32, kind="ExternalInput")
            for i in range(3)]
    bias_in = [nc.dram_tensor(f"bias{i + 1}", [P, D], _F32,
                              kind="ExternalInput") for i in range(3)]
    iota_in = nc.dram_tensor("iota", [P, P], _BF16, kind="ExternalInput")
    idx_in = nc.dram_tensor("idx16", [P, tot_idx // 16], _I16,
                            kind="ExternalInput")
    drel_in = nc.dram_tensor("dstrel", [P, tot_cols], _F32,
                             kind="ExternalInput")
    out_t = nc.dram_tensor("out", [RPC, D], _F32, kind="ExternalOutput")

    with tile.TileContext(nc) as tc:
        with tc.tile_pool(name="dram", bufs=1, space="DRAM") as dram, \
             tc.tile_pool(name="const", bufs=1) as cst, \
             tc.tile_pool(name="persist", bufs=1) as per, \
             tc.tile_pool(name="msgp", bufs=cfg.MSGBUFS) as msgp, \
             tc.tile_pool(name="idxp", bufs=cfg.MSGBUFS) as idxp, \
             tc.tile_pool(name="qp", bufs=cfg.QBUFS) as qp, \
             tc.tile_pool(name="wk", bufs=cfg.WKBUFS) as wk, \
             tc.tile_pool(name="psa", bufs=cfg.PSABUFS, space="PSUM") as psa, \
             tc.tile_pool(name="psg", bufs=2, space="PSUM") as psg, \
             tc.tile_pool(name="pst", bufs=2, space="PSUM") as pst:

            nc.gpsimd.load_library(library_config.mlp)

            g_owns = [dram.tile([RPC, 2 * D], _BF16, name=f"g_own{i}")
                      for i in range(cfg.LAYERS)]
            g_fulls = [dram.tile([GR, 2 * D], _BF16, addr_space="Shared",
                                 name=f"g_full{i}")
                       for i in range(cfg.LAYERS)]

            iota = cst.tile([P, P], _BF16)
            nc.sync.dma_start(iota[:], iota_in[:])
            ident = cst.tile([P, P], _F32)
            make_identity(nc, ident[:])
            dinv1 = cst.tile([P, TILES], _F32)
            nc.sync.dma_start(dinv1[:], dinv1_in[:])
            dinv2 = cst.tile([P, TILES], _F32)
            nc.sync.dma_start(dinv2[:], dinv2_in[:])
            ws, bs = [], []
            for i in range(3):
                w = cst.tile([D, D], _F32, name=f"w{i}")
                nc.sync.dma_start(w[:], w_in[i][:])
                ws.append(w)
                bt = cst.tile([P, D], _F32, name=f"b{i}")
                nc.sync.dma_start(bt[:], bias_in[i][:])
                bs.append(bt)

            zcol = cst.tile([P, D], _BF16)
            nc.gpsimd.memset(zcol[:], 0.0)
            ht = per.tile([D, RPC], _F32)          # H.T (current layer input)
            nc.sync.dma_start(ht[:], xT_in[:])
            g2b = per.tile([P, TILES * D], _F32)   # dinv^2*(H@W) + bias

            for L in range(cfg.LAYERS):
                g_own = g_owns[L]
                g_full = g_fulls[L]
                # ---------- phase A: G = dinv * (H @ W) ----------
                for t in range(TILES):
                    pg = psg.tile([P, D], _F32, tag="pg", name=f"pg{L}_{t}")
                    nc.tensor.matmul(
                        pg[:], lhsT=ht[:, t * P:(t + 1) * P],
                        rhs=ws[L % 3][:], start=True, stop=True,
                    )
                    g = wk.tile([P, 2 * D], _BF16, tag="g", name=f"g{L}_{t}")
                    nc.vector.tensor_scalar_mul(g[:, :D], pg[:],
                                                dinv1[:, t:t + 1])
                    nc.vector.tensor_copy(g[:, D:], zcol[:])
                    nc.sync.dma_start(g_own[t * P:(t + 1) * P, :], g[:])
                    g2 = wk.tile([P, D], _F32, tag="g2", name=f"g2_{L}_{t}")
                    nc.vector.tensor_scalar_mul(g2[:], pg[:],
                                                dinv2[:, t:t + 1])
                    nc.vector.tensor_tensor(
                        out=g2b[:, t * D:(t + 1) * D], in0=g2[:],
                        in1=bs[L % 3][:], op=mybir.AluOpType.add,
                    )

                # ---------- phase B: AllGather ----------
                if "ag" not in cfg.ABLATE:
                    nc.gpsimd.collective_compute(
                        "AllGather",
                        mybir.AluOpType.bypass,
                        replica_groups=[list(range(CORES))],
                        ins=[g_own[:]],
                        outs=[g_full[:]],
                    )

                # ---------- phase C: edge aggregation ----------
                for gi_i, gi in enumerate(ginfos):
                    blocks = gi["blocks"]
                    i0 = gi["blk0"] * P
                    nidx_g = blocks * P
                    msg = msgp.tile([P, gblk_max, 2 * D], _BF16, tag="msg",
                                    name=f"msg{L}_{gi_i}",
                                    padded_shape=[P, gblk_max, 2 * D])
                    idx_sb = idxp.tile([P, (gblk_max * P) // 16], _I16,
                                       tag="idx", name=f"idx{L}_{gi_i}",
                                       padded_shape=[P, (gblk_max * P) // 16])
                    drel_sb = idxp.tile([P, gcol_max], _F32, tag="drel",
                                        name=f"drel{L}_{gi_i}",
                                        padded_shape=[P, gcol_max])
                    nc.sync.dma_start(
                        idx_sb[:, :nidx_g // 16],
                        idx_in[:, i0 // 16:(i0 + nidx_g) // 16],
                    )
                    nc.sync.dma_start(
                        drel_sb[:, :gi["ncols"]],
                        drel_in[:, gi["col0"]:gi["col0"] + gi["ncols"]],
                    )
                    for b, (bo, kgb) in gi["spans"].items():
                        if "gather" in cfg.ABLATE:
                            continue
                        # single_packet=True caps a call at 64 descriptors
                        # (~1024 idx); single_packet=False has no such cap
                        for s0 in range(0, kgb, cfg.GCAP):
                            kk = min(cfg.GCAP, kgb - s0)
                            bo2 = bo + s0
                            nidx = kk * P
                            nc.gpsimd.dma_gather(
                                msg[:, bo2:bo2 + kk, :],
                                g_full[b * BUCK_ROWS:(b + 1) * BUCK_ROWS, :],
                                idx_sb[:, bo2 * P // 16:
                                       (bo2 * P + nidx) // 16],
                                nidx, nidx, 2 * D,
                                single_packet=cfg.SPACK,
                            )

                    for t in gi["tiles"]:
                        tbs = [(b, gi["tb"][(t, b)]) for b in range(NBUCK)
                               if (t, b) in gi["tb"]]
                        kt = sum(tb[1][1] for tb in tbs)
                        if "mm" in cfg.ABLATE:
                            tbs = []
                            kt = 0
                        pa = (psa.tile([P, D], _F32, tag="pa",
                                       name=f"pa{L}_{t}") if kt else None)
                        first = True
                        for bi, (b, (jb, njt, colg, so)) in enumerate(tbs):
                            q = qp.tile([P, njt, P], _BF16, tag="q",
                                        name=f"q{L}_{t}_{b}",
                                        padded_shape=[P, kmaxb, P])
                            if "q" not in cfg.ABLATE:
                                for j in range(njt):
                                    nc.vector.tensor_scalar(
                                        out=q[:, j, :], in0=iota[:],
                                        scalar1=drel_sb[:, colg + j:
                                                        colg + j + 1],
                                        scalar2=None,
                                        op0=mybir.AluOpType.is_equal,
                                    )
                            for j in range(njt):
                                nc.tensor.matmul(
                                    pa[:], lhsT=q[:, j, :],
                                    rhs=msg[:, jb + j, :D],
                                    start=first,
                                    stop=(bi == len(tbs) - 1 and
                                          j == njt - 1),
                                )
                                first = False
                        v = wk.tile([P, D], _F32, tag="v", name=f"v{L}_{t}")
                        if kt > 0:
                            nc.vector.tensor_scalar_mul(
                                v[:], pa[:], dinv1[:, t:t + 1])
                            nc.vector.tensor_tensor(
                                out=v[:], in0=v[:],
                                in1=g2b[:, t * D:(t + 1) * D],
                                op=mybir.AluOpType.add)
                        else:
                            nc.vector.tensor_copy(
                                v[:], g2b[:, t * D:(t + 1) * D])
                        hm = wk.tile([P, D], _F32, tag="hm", name=f"hm{L}_{t}")
                        nc.vector.tensor_scalar_mul(hm[:], v[:], NEG_SLOPE)
                        h = wk.tile([P, D], _F32, tag="h", name=f"h{L}_{t}")
                        nc.vector.tensor_tensor(
                            out=h[:], in0=hm[:], in1=v[:],
                            op=mybir.AluOpType.max)
                        if L < cfg.LAYERS - 1:
                            pt = pst.tile([D, P], _F32, tag="pt",
                                          name=f"pt{L}_{t}")
                            nc.tensor.transpose(pt[:], h[:], ident[:])
                            nc.vector.tensor_copy(ht[:, t * P:(t + 1) * P],
                                                  pt[:])
                        else:
                            nc.sync.dma_start(out_t[t * P:(t + 1) * P, :],
                                              h[:])

    nc.compile()
    return nc


def make_in_maps(x, Ws, bss, meta, per_core, cfg):
    dinv = meta["dinv"]
    CORES, NPC, RPC, TILES = cfg.CORES, cfg.NPC, cfg.RPC, cfg.TILES
    import ml_dtypes
    iota_np = np.broadcast_to(np.arange(P).astype(ml_dtypes.bfloat16),
                              (P, P)).copy()
    in_maps = []
    for c in range(CORES):
        sl = slice(c * NPC, (c + 1) * NPC)
        xT = np.zeros((D, RPC), np.float32)
        xT[:, :NPC] = x[sl].T
        d1c = np.zeros(RPC, np.float32)
        d1c[:NPC] = dinv[sl]
        d1 = d1c.reshape(TILES, P).T.copy()
        d2 = (d1 * d1).astype(np.float32)
        im = {
            "xT": xT,
            "dinv1": d1,
            "dinv2": d2,
            "iota": iota_np,
            "idx16": per_core[c]["idx16"],
            "dstrel": per_core[c]["dstrel"],
        }
        for i in range(3):
            im[f"W{i + 1}"] = Ws[i]
            im[f"bias{i + 1}"] = np.broadcast_to(
                bss[i], (P, D)).astype(np.float32).copy()
        in_maps.append(im)
    return in_maps


_CACHE = {}


def kernel(x, edge_index, W1, b1, W2, b2, W3, b3):
    cfg = DEFAULT_CFG
    x = np.asarray(x, dtype=np.float32)
    Ws = [np.asarray(w, dtype=np.float32) for w in (W1, W2, W3)]
    bss = [np.asarray(b, dtype=np.float32) for b in (b1, b2, b3)]

    ei = np.asarray(edge_index)
    key = hash(ei[:, ::997].tobytes()) ^ hash(ei.shape)
    if key not in _CACHE:
        meta, per_core = _preprocess(ei, cfg)
        nc = _build_program(meta, cfg)
        _CACHE[key] = (meta, per_core, nc)
    meta, per_core, nc = _CACHE[key]

    in_maps = make_in_maps(x, Ws, bss, meta, per_core, cfg)
    res = run_bass_kernel_spmd(nc, in_maps, core_ids=list(range(cfg.CORES)))
    out = np.empty((cfg.N, D), np.float32)
    for c in range(cfg.CORES):
        out[c * cfg.NPC:(c + 1) * cfg.NPC] = res.results[c]["out"][:cfg.NPC]
    return out



# revision 4
# speedup vs baseline: 1.0023x; 1.0023x over previous
"""3-layer GCN (GCNConv x3 + LeakyReLU, PyG semantics) on 8 Trainium2 cores.

Strategy (source-partitioned, ReduceScatter):
  - Core c owns nodes [c*NPC, (c+1)*NPC) and processes the edges whose SOURCE
    it owns.  Self loops are added as explicit edges, so a whole layer is
        OUT[d] = lrelu(dinv[d] * sum_{e: dst=d} G[src_e] + bias),
    with G = dinv * (H @ W) computed locally per core (no feature exchange).
  - Per layer: phase A computes G for own nodes into two local DRAM gather
    tables (low/high half of own rows, int16-indexable, 256B rows), then
    dma_gather over globally dst-sorted edges + one-hot matmul segment-sum
    into PSUM per global dst tile (784 tiles across all cores), converted to
    bf16 and written into a row-major [100352, 64] partial-sum table.
  - One ReduceScatter(add) delivers each core the full sum for its own
    12544 rows -- an output-sized collective (~57us) instead of AllGathering
    the whole feature table (~284us).
  - Epilogue per own tile: scale by dinv (ACT), add bias (DVE), LeakyReLU
    (ACT), PE transpose, and the next layer's phase-A matmul (bf16).
  - One-hot q matrices (is_equal(iota, drel)) are split across DVE and Pool;
    PSUM->bf16 converts across ACT/DVE, keeping every engine below the DMA
    roofline.  Gathers use one large call per (gather-group, table-half)
    span to amortize the ~1us SWDGE fixed cost per call.

The Bass program is SPMD: one program, per-core data.  Section lengths are
shared across cores (max over cores, padded with dummy edges whose one-hot
column is zero: dstrel = -1).
"""
import sys

sys.path.insert(0, "/opt/trn_rl_repo")

import numpy as np

import concourse.bacc as bacc
import concourse.mybir as mybir
import concourse.tile as tile
from concourse import library_config
from concourse.bass_utils import run_bass_kernel_spmd
from concourse.masks import make_identity

_F32 = mybir.dt.float32
_BF16 = mybir.dt.bfloat16
_I16 = mybir.dt.int16
P = 128
D = 64
NEG_SLOPE = 0.01


class Cfg:
    def __init__(self, n_nodes=100000, cores=8):
        self.N = n_nodes
        self.CORES = cores
        self.NPC = self.N // cores            # nodes owned per core
        self.TILES = (self.NPC + P - 1) // P  # own-node tiles per core (98)
        self.RPC = self.TILES * P             # padded rows per core (12544)
        self.GR = cores * self.RPC            # global padded rows (100352)
        self.GTILES = cores * self.TILES      # global dst tiles (784)
        self.KB = 2                           # source buckets (table halves)
        assert self.RPC % self.KB == 0
        self.HALF = self.RPC // self.KB       # rows per gather table (6272)
        assert self.HALF <= 32767
        self.GGT = 28                         # dst tiles per gather group
        assert self.GTILES % self.GGT == 0
        self.NGG = self.GTILES // self.GGT    # gather groups (28)
        self.GT = 7                           # dst tiles per write group
        assert self.GGT % self.GT == 0
        self.WPG = self.GGT // self.GT        # write groups per gather group
        self.HTILES = self.TILES // self.KB   # own tiles per table half (49)
        assert self.TILES % self.KB == 0
        self.LAYERS = 3
        self.MSGBUFS = 2
        self.QBUFS = 6
        self.PSABUFS = 4
        # engine split knobs
        self.Q_SPLIT = 3      # every Q_SPLIT-th q op goes to Pool
        self.CONV_MOD = 4     # converts: i%MOD==0 -> DVE, else ACT


DEFAULT_CFG = Cfg()


def _preprocess(edge_index, cfg):
    """Sort/pack edges; build per-core device arrays and shared metadata."""
    src0 = np.asarray(edge_index[0], dtype=np.int64)
    dst0 = np.asarray(edge_index[1], dtype=np.int64)
    N, CORES, NPC, TILES = cfg.N, cfg.CORES, cfg.NPC, cfg.TILES
    NGG, KB, GGT, HALF = cfg.NGG, cfg.KB, cfg.GGT, cfg.HALF

    src = src0
    dst = dst0

    deg = np.bincount(dst, minlength=N).astype(np.float32) + 1.0
    dinv = (1.0 / np.sqrt(deg)).astype(np.float32)

    owner = src // NPC                      # processing core (source owner)
    slocal = src % NPC
    bucket = slocal // HALF                 # gather-table half
    lrow = (slocal % HALF).astype(np.int16)

    downer = dst // NPC
    dloc = dst % NPC
    gtile = downer * TILES + dloc // P      # global dst tile 0..783
    drel_v = (dloc % P).astype(np.float32)
    grp = gtile // GGT
    tl = gtile % GGT

    counts = np.zeros((CORES, NGG, KB, GGT), dtype=np.int64)
    np.add.at(counts, (owner, grp, bucket, tl), 1)
    order = np.lexsort((tl, bucket, grp, owner))
    s_lrow = lrow[order]
    s_drel = drel_v[order]

    sec_len = counts.max(axis=0)            # [NGG, KB, GGT] shared sections

    # layout: gather group -> bucket span (padded to x128) -> tile sections
    ginfos = []
    tot_blocks = 0
    tot_cols = 0
    for g in range(NGG):
        gi = {"blk0": tot_blocks, "col0": tot_cols, "spans": {}, "tb": {}}
        gblk = 0
        gcol = 0
        for b in range(KB):
            span_len = int(sec_len[g, b].sum())
            if span_len == 0:
                continue
            kgb = -(-span_len // P)
            gi["spans"][b] = (gblk, kgb)
            so = 0
            for t in range(GGT):
                stb = int(sec_len[g, b, t])
                if stb == 0:
                    continue
                j0 = so // P
                njt = (so + stb - 1) // P - j0 + 1
                gi["tb"][(b, t)] = (gblk + j0, njt, gcol, so)
                gcol += njt
                so += stb
            gblk += kgb
        gi["blocks"] = gblk
        gi["ncols"] = gcol
        tot_blocks += gblk
        tot_cols += gcol
        ginfos.append(gi)
    tot_idx = tot_blocks * P

    # per-core run starts in the sorted edge list ((c, g, b, t)-major order)
    flat = counts.reshape(-1)
    starts = np.zeros(flat.size, dtype=np.int64)
    starts[1:] = np.cumsum(flat)[:-1]
    starts = starts.reshape(CORES, NGG, KB, GGT)

    per_core = []
    for c in range(CORES):
        lidx_flat = np.zeros(tot_idx, dtype=np.int16)
        drel = np.full((P, tot_cols), -1.0, dtype=np.float32)
        for g in range(NGG):
            gi = ginfos[g]
            for b, (bo, kgb) in gi["spans"].items():
                span_i0 = (gi["blk0"] + bo) * P
                for t in range(GGT):
                    if (b, t) not in gi["tb"]:
                        continue
                    jb, njt, colg, so = gi["tb"][(b, t)]
                    n = int(counts[c, g, b, t])
                    if n == 0:
                        continue
                    s0 = int(starts[c, g, b, t])
                    pos0 = span_i0 + so
                    lidx_flat[pos0:pos0 + n] = s_lrow[s0:s0 + n]
                    q = so + np.arange(n)
                    jrel = q // P - so // P
                    pp = (pos0 + np.arange(n)) % P
                    cols = gi["col0"] + colg + jrel
                    drel[pp, cols] = s_drel[s0:s0 + n]
        idx16 = np.tile(lidx_flat.reshape(tot_idx // 16, 16).T, (8, 1)).copy()
        per_core.append({"idx16": idx16, "dstrel": drel})

    meta = {
        "ginfos": ginfos,
        "tot_idx": tot_idx,
        "tot_cols": tot_cols,
        "dinv": dinv,
    }
    return meta, per_core


def _build_program(meta, cfg):
    ginfos = meta["ginfos"]
    tot_idx = meta["tot_idx"]
    tot_cols = meta["tot_cols"]
    CORES, TILES, RPC = cfg.CORES, cfg.TILES, cfg.RPC
    NGG, KB, GGT, GT = cfg.NGG, cfg.KB, cfg.GGT, cfg.GT
    HALF, HTILES, WPG, GR = cfg.HALF, cfg.HTILES, cfg.WPG, cfg.GR

    kmax_g = max(gi["blocks"] for gi in ginfos)

    nc = bacc.Bacc("TRN2", debug=False)
    nc.num_devices = CORES

    xT_in = nc.dram_tensor("xT", [D, RPC], _BF16, kind="ExternalInput")
    dinv1_in = nc.dram_tensor("dinv1", [P, TILES], _F32, kind="ExternalInput")
    dinv2_in = nc.dram_tensor("dinv2", [P, TILES], _F32, kind="ExternalInput")
    w_in = [nc.dram_tensor(f"W{i + 1}", [D, D], _F32, kind="ExternalInput")
            for i in range(3)]
    bias_in = [nc.dram_tensor(f"bias{i + 1}", [P, D], _F32,
                              kind="ExternalInput") for i in range(3)]
    iota_in = nc.dram_tensor("iota", [P, P], _BF16, kind="ExternalInput")
    idx_in = nc.dram_tensor("idx16", [P, tot_idx // 16], _I16,
                            kind="ExternalInput")
    drel_in = nc.dram_tensor("dstrel", [P, tot_cols], _F32,
                             kind="ExternalInput")
    out_t = nc.dram_tensor("out", [RPC, D], _F32, kind="ExternalOutput")

    with tile.TileContext(nc) as tc:
        with tc.tile_pool(name="dram", bufs=1, space="DRAM") as dram, \
             tc.tile_pool(name="const", bufs=1) as cst, \
             tc.tile_pool(name="msgp", bufs=cfg.MSGBUFS) as msgp, \
             tc.tile_pool(name="qp", bufs=cfg.QBUFS) as qp, \
             tc.tile_pool(name="gsp", bufs=2) as gsp, \
             tc.tile_pool(name="psp", bufs=3) as psp, \
             tc.tile_pool(name="rsp", bufs=2) as rsp, \
             tc.tile_pool(name="wk", bufs=4) as wk, \
             tc.tile_pool(name="htp", bufs=4) as htpp, \
             tc.tile_pool(name="psa", bufs=cfg.PSABUFS, space="PSUM") as psa, \
             tc.tile_pool(name="psg", bufs=2, space="PSUM") as psg, \
             tc.tile_pool(name="pst", bufs=2, space="PSUM") as pst:

            nc.gpsimd.load_library(library_config.mlp)

            gtabs = [[dram.tile([HALF, 2 * D], _BF16, name=f"gt{L}_{b}")
                      for b in range(KB)] for L in range(cfg.LAYERS)]
            partials = [dram.tile([GR, D], _BF16, name=f"part{L}")
                        for L in range(cfg.LAYERS)]
            rsouts = [dram.tile([RPC, D], _BF16, name=f"rsout{L}")
                      for L in range(cfg.LAYERS)]

            iota = cst.tile([P, P], _BF16)
            nc.sync.dma_start(iota[:], iota_in[:])
            ident = cst.tile([P, P], _F32)
            make_identity(nc, ident[:])
            dinv1 = cst.tile([P, TILES], _F32)
            nc.sync.dma_start(dinv1[:], dinv1_in[:])
            dinv2 = cst.tile([P, TILES], _F32)
            nc.sync.dma_start(dinv2[:], dinv2_in[:])
            g2b = cst.tile([P, TILES * D], _BF16)
            ws, bs = [], []
            for i in range(3):
                w = cst.tile([D, D], _F32, name=f"w{i}")
                nc.sync.dma_start(w[:], w_in[i][:])
                wb = cst.tile([D, D], _BF16, name=f"wb{i}")
                nc.vector.tensor_copy(wb[:], w[:])
                ws.append(wb)
                bt = cst.tile([P, D], _F32, name=f"b{i}")
                nc.sync.dma_start(bt[:], bias_in[i][:])
                bs.append(bt)
            idx_sb = cst.tile([P, tot_idx // 16], _I16)
            nc.sync.dma_start(idx_sb[:], idx_in[:])
            drel_sb = cst.tile([P, tot_cols], _F32)
            nc.sync.dma_start(drel_sb[:], drel_in[:])

            # zero upper halves of the gather tables (once)
            zstage = cst.tile([P, HTILES * D], _BF16)
            nc.gpsimd.memset(zstage[:], 0.0)
            for L in range(cfg.LAYERS):
                for b in range(KB):
                    dst = gtabs[L][b][:, D:2 * D].rearrange(
                        "(j p) c -> p j c", j=HTILES, p=P)
                    nc.sync.dma_start(dst, zstage[:].rearrange(
                        "p (j c) -> p j c", j=HTILES, c=D))

            ht0 = cst.tile([D, RPC], _BF16)        # layer-0 input (x.T)
            nc.sync.dma_start(ht0[:], xT_in[:])

            hstage = cst.tile([P, TILES * D], _F32)

            qctr = [0]
            cctr = [0]

            def build_q(qt, col):
                eng = (nc.gpsimd if (qctr[0] % cfg.Q_SPLIT == cfg.Q_SPLIT - 1)
                       else nc.vector)
                qctr[0] += 1
                eng.tensor_scalar(
                    out=qt[:], in0=iota[:],
                    scalar1=drel_sb[:, col:col + 1], scalar2=None,
                    op0=mybir.AluOpType.is_equal)

            def convert(dst_ap, src_ap):
                m = cctr[0] % cfg.CONV_MOD
                cctr[0] += 1
                if m == 0:
                    nc.vector.tensor_copy(dst_ap, src_ap)
                else:
                    nc.scalar.copy(dst_ap, src_ap)

            def emit_gathers(L, g):
                gi = ginfos[g]
                msg = msgp.tile([P, gi["blocks"], 2 * D], _BF16,
                                tag="msg", name=f"msg{L}_{g}",
                                padded_shape=[P, kmax_g, 2 * D])
                for b, (bo, kgb) in gi["spans"].items():
                    i0 = (gi["blk0"] + bo) * P
                    nidx = kgb * P
                    nc.gpsimd.dma_gather(
                        msg[:, bo:bo + kgb, :],
                        gtabs[L][b][:],
                        idx_sb[:, i0 // 16:(i0 + nidx) // 16],
                        nidx, nidx, 2 * D,
                        single_packet=False)
                return msg

            def emit_process(L, g, msg):
                gi = ginfos[g]
                for wg in range(WPG):
                    pstage = psp.tile([P, GT * D], _BF16, tag="ps",
                                      name=f"ps{L}_{g}_{wg}")
                    for tw in range(GT):
                        t = wg * GT + tw
                        tbs = [(b, gi["tb"][(b, t)]) for b in range(KB)
                               if (b, t) in gi["tb"]]
                        nmm = sum(e[1][1] for e in tbs)
                        assert nmm > 0
                        pa = psa.tile([P, D], _F32, tag="pa",
                                      name=f"pa{L}_{g}_{t}")
                        done = 0
                        for b, (jb, njt, colg, so) in tbs:
                            for j in range(njt):
                                qt = qp.tile([P, P], _BF16, tag="q",
                                             name=f"q{L}_{g}_{t}_{b}_{j}")
                                build_q(qt, gi["col0"] + colg + j)
                                nc.tensor.matmul(
                                    pa[:], lhsT=qt[:],
                                    rhs=msg[:, jb + j, :D],
                                    start=(done == 0),
                                    stop=(done == nmm - 1))
                                done += 1
                        convert(pstage[:, tw * D:(tw + 1) * D], pa[:])
                    r0 = (g * GGT + wg * GT) * P
                    dst = partials[L][r0:r0 + GT * P, :].rearrange(
                        "(j p) c -> p j c", j=GT, p=P)
                    nc.sync.dma_start(dst, pstage[:].rearrange(
                        "p (j c) -> p j c", j=GT, c=D))

            def emit_phase_a(L):
                """Epilogue of layer L-1 (if L>0) fused with phase A of L."""
                rsall = None
                if L > 0:
                    rsall = rsp.tile([P, TILES * D], _BF16, tag="rs",
                                     name=f"rsall{L}")
                    src = rsouts[L - 1][:].rearrange(
                        "(j p) c -> p j c", j=TILES, p=P)
                    nc.sync.dma_start(rsall[:].rearrange(
                        "p (j c) -> p j c", j=TILES, c=D), src)
                for b in range(KB):
                    gstage = gsp.tile([P, HTILES * D], _BF16, tag="gs",
                                      name=f"gs{L}_{b}")
                    for th in range(HTILES):
                        t = b * HTILES + th
                        if L == 0:
                            lhs = ht0[:, t * P:(t + 1) * P]
                        else:
                            v = wk.tile([P, D], _BF16, tag="v",
                                        name=f"v{L}_{t}")
                            nc.scalar.mul(v[:], rsall[:, t * D:(t + 1) * D],
                                          dinv1[:, t:t + 1])
                            nc.vector.tensor_tensor(
                                out=v[:], in0=v[:],
                                in1=g2b[:, t * D:(t + 1) * D],
                                op=mybir.AluOpType.add)
                            h = wk.tile([P, D], _F32, tag="h",
                                        name=f"h{L}_{t}")
                            nc.scalar.activation(
                                h[:], v[:],
                                mybir.ActivationFunctionType.Lrelu,
                                bias=0.0, scale=1.0, alpha=NEG_SLOPE)
                            pt = pst.tile([D, P], _F32, tag="pt",
                                          name=f"pt{L}_{t}")
                            nc.tensor.transpose(pt[:], h[:], ident[:])
                            htp = htpp.tile([D, P], _BF16, tag="ht",
                                            name=f"htp{L}_{t}")
                            nc.vector.tensor_copy(htp[:], pt[:])
                            lhs = htp[:]
                        pg = psg.tile([P, D], _F32, tag="pg",
                                      name=f"pg{L}_{t}")
                        nc.tensor.matmul(pg[:], lhsT=lhs, rhs=ws[L][:],
                                         start=True, stop=True)
                        nc.vector.tensor_scalar_mul(
                            gstage[:, th * D:(th + 1) * D], pg[:],
                            dinv1[:, t:t + 1])
                        g2 = wk.tile([P, D], _F32, tag="g2",
                                     name=f"g2_{L}_{t}")
                        nc.vector.tensor_scalar_mul(g2[:], pg[:],
                                                    dinv2[:, t:t + 1])
                        nc.vector.tensor_tensor(
                            out=g2b[:, t * D:(t + 1) * D], in0=g2[:],
                            in1=bs[L][:], op=mybir.AluOpType.add)
                    dst = gtabs[L][b][:, :D].rearrange(
                        "(j p) c -> p j c", j=HTILES, p=P)
                    nc.sync.dma_start(dst, gstage[:].rearrange(
                        "p (j c) -> p j c", j=HTILES, c=D))

            for L in range(cfg.LAYERS):
                emit_phase_a(L)
                msgs = {0: emit_gathers(L, 0)}
                for g in range(NGG):
                    if g + 1 < NGG:
                        msgs[g + 1] = emit_gathers(L, g + 1)
                    emit_process(L, g, msgs.pop(g))
                nc.gpsimd.collective_compute(
                    "ReduceScatter",
                    mybir.AluOpType.add,
                    replica_groups=[list(range(CORES))],
                    ins=[partials[L][:]],
                    outs=[rsouts[L][:]],
                )

            # ---------- final epilogue -> output ----------
            L = cfg.LAYERS
            rsall = rsp.tile([P, TILES * D], _BF16, tag="rs", name="rsall_f")
            src = rsouts[L - 1][:].rearrange("(j p) c -> p j c", j=TILES, p=P)
            nc.sync.dma_start(rsall[:].rearrange(
                "p (j c) -> p j c", j=TILES, c=D), src)
            for t in range(TILES):
                v = wk.tile([P, D], _BF16, tag="v", name=f"vf_{t}")
                nc.scalar.mul(v[:], rsall[:, t * D:(t + 1) * D],
                              dinv1[:, t:t + 1])
                nc.vector.tensor_tensor(
                    out=v[:], in0=v[:],
                    in1=g2b[:, t * D:(t + 1) * D],
                    op=mybir.AluOpType.add)
                nc.scalar.activation(
                    hstage[:, t * D:(t + 1) * D], v[:],
                    mybir.ActivationFunctionType.Lrelu,
                    bias=0.0, scale=1.0, alpha=NEG_SLOPE)
            dst = out_t[:].rearrange("(j p) c -> p j c", j=TILES, p=P)
            nc.sync.dma_start(dst, hstage[:].rearrange(
                "p (j c) -> p j c", j=TILES, c=D))

    nc.compile()
    return nc


def make_in_maps(x, Ws, bss, meta, per_core, cfg):
    dinv = meta["dinv"]
    CORES, NPC, RPC, TILES = cfg.CORES, cfg.NPC, cfg.RPC, cfg.TILES
    import ml_dtypes
    iota_np = np.broadcast_to(np.arange(P).astype(ml_dtypes.bfloat16),
                              (P, P)).copy()
    in_maps = []
    for c in range(CORES):
        sl = slice(c * NPC, (c + 1) * NPC)
        xT = np.zeros((D, RPC), np.float32)
        xT[:, :NPC] = x[sl].T
        d1c = np.zeros(RPC, np.float32)
        d1c[:NPC] = dinv[sl]
        d1 = d1c.reshape(TILES, P).T.copy()
        d2 = (d1 * d1).astype(np.float32)
        im = {
            "xT": xT.astype(ml_dtypes.bfloat16),
            "dinv1": d1,
            "dinv2": d2,
            "iota": iota_np,
            "idx16": per_core[c]["idx16"],
            "dstrel": per_core[c]["dstrel"],
        }
        for i in range(3):
            im[f"W{i + 1}"] = Ws[i]
            im[f"bias{i + 1}"] = np.broadcast_to(
                bss[i], (P, D)).astype(np.float32).copy()
        in_maps.append(im)
    return in_maps


_CACHE = {}


def kernel(x, edge_index, W1, b1, W2, b2, W3, b3):
    cfg = DEFAULT_CFG
    x = np.asarray(x, dtype=np.float32)
    Ws = [np.asarray(w, dtype=np.float32) for w in (W1, W2, W3)]
    bss = [np.asarray(b, dtype=np.float32) for b in (b1, b2, b3)]

    ei = np.asarray(edge_index)
    key = hash(ei[:, ::997].tobytes()) ^ hash(ei.shape)
    if key not in _CACHE:
        meta, per_core = _preprocess(ei, cfg)
        nc = _build_program(meta, cfg)
        _CACHE[key] = (meta, per_core, nc)
    meta, per_core, nc = _CACHE[key]

    in_maps = make_in_maps(x, Ws, bss, meta, per_core, cfg)
    res = run_bass_kernel_spmd(nc, in_maps, core_ids=list(range(cfg.CORES)))
    out = np.empty((cfg.N, D), np.float32)
    for c in range(cfg.CORES):
        out[c * cfg.NPC:(c + 1) * cfg.NPC] = res.results[c]["out"][:cfg.NPC]
    return out


# revision 7
# speedup vs baseline: 1.1340x; 1.1314x over previous
"""3-layer GCN (GCNConv x3 + LeakyReLU, PyG semantics) on 8 Trainium2 cores.

Strategy (source-partitioned, ReduceScatter):
  - Core c owns nodes [c*NPC, (c+1)*NPC) and processes the edges whose SOURCE
    it owns.  Self loops are added as explicit edges, so a whole layer is
        OUT[d] = lrelu(dinv[d] * sum_{e: dst=d} G[src_e] + bias),
    with G = dinv * (H @ W) computed locally per core (no feature exchange).
  - Per layer: phase A computes G for own nodes into two local DRAM gather
    tables (low/high half of own rows, int16-indexable, 256B rows), then
    dma_gather over globally dst-sorted edges + one-hot matmul segment-sum
    into PSUM per global dst tile (784 tiles across all cores), converted to
    bf16 and written into a row-major [100352, 64] partial-sum table.
  - One ReduceScatter(add) delivers each core the full sum for its own
    12544 rows -- an output-sized collective (~57us) instead of AllGathering
    the whole feature table (~284us).
  - Epilogue per own tile: scale by dinv (ACT), add bias (DVE), LeakyReLU
    (ACT), PE transpose, and the next layer's phase-A matmul (bf16).
  - One-hot q matrices (is_equal(iota, drel)) are split across DVE and Pool;
    PSUM->bf16 converts across ACT/DVE, keeping every engine below the DMA
    roofline.  Gathers use one large call per (gather-group, table-half)
    span to amortize the ~1us SWDGE fixed cost per call.

The Bass program is SPMD: one program, per-core data.  Section lengths are
shared across cores (max over cores, padded with dummy edges whose one-hot
column is zero: dstrel = -1).
"""
import sys

sys.path.insert(0, "/opt/trn_rl_repo")

import numpy as np

import concourse.bacc as bacc
import concourse.mybir as mybir
import concourse.tile as tile
from concourse import library_config
from concourse.bass_utils import run_bass_kernel_spmd
from concourse.masks import make_identity

_F32 = mybir.dt.float32
_BF16 = mybir.dt.bfloat16
_I16 = mybir.dt.int16
P = 128
D = 64
NEG_SLOPE = 0.01


class Cfg:
    def __init__(self, n_nodes=100000, cores=8):
        self.N = n_nodes
        self.CORES = cores
        self.NPC = self.N // cores            # nodes owned per core
        self.TILES = (self.NPC + P - 1) // P  # own-node tiles per core (98)
        self.RPC = self.TILES * P             # padded rows per core (12544)
        self.GR = cores * self.RPC            # global padded rows (100352)
        self.GTILES = cores * self.TILES      # global dst tiles (784)
        self.KB = 1                           # source buckets (gather tables)
        assert self.RPC % self.KB == 0
        self.HALF = self.RPC // self.KB       # rows per gather table (12544)
        assert self.HALF <= 32767
        self.GGT = 28                         # dst tiles per gather group
        assert self.GTILES % self.GGT == 0
        self.NGG = self.GTILES // self.GGT    # gather groups (28)
        self.GT = 7                           # dst tiles per write group
        assert self.GGT % self.GT == 0
        self.WPG = self.GGT // self.GT        # write groups per gather group
        self.HTILES = self.TILES // self.KB   # own tiles per table (98)
        assert self.TILES % self.KB == 0
        self.LAYERS = 3
        self.MSGBUFS = 3
        self.QBUFS = 6
        self.PSABUFS = 4
        # engine split knobs
        self.Q_SPLIT = 3      # every Q_SPLIT-th q op goes to Pool
        self.CONV_MOD = 4     # converts: i%MOD==0 -> DVE, else ACT


DEFAULT_CFG = Cfg()


def _preprocess(edge_index, cfg):
    """Sort/pack edges; build per-core device arrays and shared metadata."""
    src0 = np.asarray(edge_index[0], dtype=np.int64)
    dst0 = np.asarray(edge_index[1], dtype=np.int64)
    N, CORES, NPC, TILES = cfg.N, cfg.CORES, cfg.NPC, cfg.TILES
    NGG, KB, GGT, HALF = cfg.NGG, cfg.KB, cfg.GGT, cfg.HALF

    src = src0
    dst = dst0

    deg = np.bincount(dst, minlength=N).astype(np.float32) + 1.0
    dinv = (1.0 / np.sqrt(deg)).astype(np.float32)

    owner = src // NPC                      # processing core (source owner)
    slocal = src % NPC
    bucket = slocal // HALF                 # gather-table half
    lrow = (slocal % HALF).astype(np.int16)

    downer = dst // NPC
    dloc = dst % NPC
    gtile = downer * TILES + dloc // P      # global dst tile 0..783
    drel_v = (dloc % P).astype(np.float32)
    grp = gtile // GGT
    tl = gtile % GGT

    counts = np.zeros((CORES, NGG, KB, GGT), dtype=np.int64)
    np.add.at(counts, (owner, grp, bucket, tl), 1)
    order = np.lexsort((tl, bucket, grp, owner))
    s_lrow = lrow[order]
    s_drel = drel_v[order]

    sec_len = counts.max(axis=0)            # [NGG, KB, GGT] shared sections

    # layout: gather group -> bucket span (padded to x128) -> tile sections
    ginfos = []
    tot_blocks = 0
    tot_cols = 0
    for g in range(NGG):
        gi = {"blk0": tot_blocks, "col0": tot_cols, "spans": {}, "tb": {}}
        gblk = 0
        gcol = 0
        for b in range(KB):
            span_len = int(sec_len[g, b].sum())
            if span_len == 0:
                continue
            kgb = -(-span_len // P)
            gi["spans"][b] = (gblk, kgb)
            so = 0
            for t in range(GGT):
                stb = int(sec_len[g, b, t])
                if stb == 0:
                    continue
                j0 = so // P
                njt = (so + stb - 1) // P - j0 + 1
                gi["tb"][(b, t)] = (gblk + j0, njt, gcol, so)
                gcol += njt
                so += stb
            gblk += kgb
        gi["blocks"] = gblk
        gi["ncols"] = gcol
        tot_blocks += gblk
        tot_cols += gcol
        ginfos.append(gi)
    tot_idx = tot_blocks * P

    # per-core run starts in the sorted edge list ((c, g, b, t)-major order)
    flat = counts.reshape(-1)
    starts = np.zeros(flat.size, dtype=np.int64)
    starts[1:] = np.cumsum(flat)[:-1]
    starts = starts.reshape(CORES, NGG, KB, GGT)

    per_core = []
    for c in range(CORES):
        lidx_flat = np.zeros(tot_idx, dtype=np.int16)
        drel = np.full((P, tot_cols), -1.0, dtype=np.float32)
        for g in range(NGG):
            gi = ginfos[g]
            for b, (bo, kgb) in gi["spans"].items():
                span_i0 = (gi["blk0"] + bo) * P
                for t in range(GGT):
                    if (b, t) not in gi["tb"]:
                        continue
                    jb, njt, colg, so = gi["tb"][(b, t)]
                    n = int(counts[c, g, b, t])
                    if n == 0:
                        continue
                    s0 = int(starts[c, g, b, t])
                    pos0 = span_i0 + so
                    lidx_flat[pos0:pos0 + n] = s_lrow[s0:s0 + n]
                    q = so + np.arange(n)
                    jrel = q // P - so // P
                    pp = (pos0 + np.arange(n)) % P
                    cols = gi["col0"] + colg + jrel
                    drel[pp, cols] = s_drel[s0:s0 + n]
        idx16 = np.tile(lidx_flat.reshape(tot_idx // 16, 16).T, (8, 1)).copy()
        per_core.append({"idx16": idx16, "dstrel": drel})

    meta = {
        "ginfos": ginfos,
        "tot_idx": tot_idx,
        "tot_cols": tot_cols,
        "dinv": dinv,
    }
    return meta, per_core


def _build_program(meta, cfg):
    ginfos = meta["ginfos"]
    tot_idx = meta["tot_idx"]
    tot_cols = meta["tot_cols"]
    CORES, TILES, RPC = cfg.CORES, cfg.TILES, cfg.RPC
    NGG, KB, GGT, GT = cfg.NGG, cfg.KB, cfg.GGT, cfg.GT
    HALF, HTILES, WPG, GR = cfg.HALF, cfg.HTILES, cfg.WPG, cfg.GR

    kmax_g = max(gi["blocks"] for gi in ginfos)

    nc = bacc.Bacc("TRN2", debug=False)
    nc.num_devices = CORES

    xT_in = nc.dram_tensor("xT", [D, RPC], _BF16, kind="ExternalInput")
    dinv1_in = nc.dram_tensor("dinv1", [P, TILES], _F32, kind="ExternalInput")
    dinv2_in = nc.dram_tensor("dinv2", [P, TILES], _F32, kind="ExternalInput")
    w_in = [nc.dram_tensor(f"W{i + 1}", [D, D], _F32, kind="ExternalInput")
            for i in range(3)]
    bias_in = [nc.dram_tensor(f"bias{i + 1}", [P, D], _F32,
                              kind="ExternalInput") for i in range(3)]
    iota_in = nc.dram_tensor("iota", [P, P], _BF16, kind="ExternalInput")
    idx_in = nc.dram_tensor("idx16", [P, tot_idx // 16], _I16,
                            kind="ExternalInput")
    drel_in = nc.dram_tensor("dstrel", [P, tot_cols], _F32,
                             kind="ExternalInput")
    out_t = nc.dram_tensor("out", [RPC, D], _F32, kind="ExternalOutput")

    with tile.TileContext(nc) as tc:
        with tc.tile_pool(name="dram", bufs=1, space="DRAM") as dram, \
             tc.tile_pool(name="const", bufs=1) as cst, \
             tc.tile_pool(name="msgp", bufs=cfg.MSGBUFS) as msgp, \
             tc.tile_pool(name="qp", bufs=cfg.QBUFS) as qp, \
             tc.tile_pool(name="gsp", bufs=2) as gsp, \
             tc.tile_pool(name="psp", bufs=3) as psp, \
             tc.tile_pool(name="rsp", bufs=1) as rsp, \
             tc.tile_pool(name="wk", bufs=4) as wk, \
             tc.tile_pool(name="htp", bufs=4) as htpp, \
             tc.tile_pool(name="psa", bufs=cfg.PSABUFS, space="PSUM") as psa, \
             tc.tile_pool(name="psg", bufs=2, space="PSUM") as psg, \
             tc.tile_pool(name="pst", bufs=2, space="PSUM") as pst:

            nc.gpsimd.load_library(library_config.mlp)

            gtabs = [[dram.tile([HALF, 2 * D], _BF16, name=f"gt{L}_{b}")
                      for b in range(KB)] for L in range(cfg.LAYERS)]
            partials = [dram.tile([GR, D], _BF16, name=f"part{L}")
                        for L in range(cfg.LAYERS)]
            rsouts = [dram.tile([RPC, D], _BF16, name=f"rsout{L}")
                      for L in range(cfg.LAYERS)]

            iota = cst.tile([P, P], _BF16)
            nc.sync.dma_start(iota[:], iota_in[:])
            ident = cst.tile([P, P], _F32)
            make_identity(nc, ident[:])
            dinv1 = cst.tile([P, TILES], _F32)
            nc.sync.dma_start(dinv1[:], dinv1_in[:])
            dinv2 = cst.tile([P, TILES], _F32)
            nc.sync.dma_start(dinv2[:], dinv2_in[:])
            g2b = cst.tile([P, TILES * D], _BF16)
            ws, bs = [], []
            for i in range(3):
                w = cst.tile([D, D], _F32, name=f"w{i}")
                nc.sync.dma_start(w[:], w_in[i][:])
                wb = cst.tile([D, D], _BF16, name=f"wb{i}")
                nc.vector.tensor_copy(wb[:], w[:])
                ws.append(wb)
                bt = cst.tile([P, D], _F32, name=f"b{i}")
                nc.sync.dma_start(bt[:], bias_in[i][:])
                bs.append(bt)
            ht0 = cst.tile([D, RPC], _BF16)        # layer-0 input (x.T)
            nc.sync.dma_start(ht0[:], xT_in[:])

            # zero upper halves of the gather tables (once)
            zstage = cst.tile([P, HTILES * D], _BF16)
            nc.gpsimd.memset(zstage[:], 0.0)
            for L in range(cfg.LAYERS):
                for b in range(KB):
                    dst = gtabs[L][b][:, D:2 * D].rearrange(
                        "(j p) c -> p j c", j=HTILES, p=P)
                    nc.sync.dma_start(dst, zstage[:].rearrange(
                        "p (j c) -> p j c", j=HTILES, c=D))

            idx_sb = cst.tile([P, tot_idx // 16], _I16)
            nc.sync.dma_start(idx_sb[:], idx_in[:])
            drel_sb = cst.tile([P, tot_cols], _F32)
            nc.sync.dma_start(drel_sb[:], drel_in[:])

            hstage = cst.tile([P, TILES * D], _F32)

            qctr = [0]
            cctr = [0]

            def build_q(qt, col):
                eng = (nc.gpsimd if (qctr[0] % cfg.Q_SPLIT == cfg.Q_SPLIT - 1)
                       else nc.vector)
                qctr[0] += 1
                eng.tensor_scalar(
                    out=qt[:], in0=iota[:],
                    scalar1=drel_sb[:, col:col + 1], scalar2=None,
                    op0=mybir.AluOpType.is_equal)

            def convert(dst_ap, src_ap):
                m = cctr[0] % cfg.CONV_MOD
                cctr[0] += 1
                if m == 0:
                    nc.vector.tensor_copy(dst_ap, src_ap)
                else:
                    nc.scalar.copy(dst_ap, src_ap)

            def emit_gathers(L, g):
                gi = ginfos[g]
                msg = msgp.tile([P, gi["blocks"], 2 * D], _BF16,
                                tag="msg", name=f"msg{L}_{g}",
                                padded_shape=[P, kmax_g, 2 * D])
                for b, (bo, kgb) in gi["spans"].items():
                    i0 = (gi["blk0"] + bo) * P
                    nidx = kgb * P
                    nc.gpsimd.dma_gather(
                        msg[:, bo:bo + kgb, :],
                        gtabs[L][b][:],
                        idx_sb[:, i0 // 16:(i0 + nidx) // 16],
                        nidx, nidx, 2 * D,
                        single_packet=False)
                return msg

            def emit_process(L, g, msg):
                gi = ginfos[g]
                for wg in range(WPG):
                    pstage = psp.tile([P, GT * D], _BF16, tag="ps",
                                      name=f"ps{L}_{g}_{wg}")
                    for tw in range(GT):
                        t = wg * GT + tw
                        tbs = [(b, gi["tb"][(b, t)]) for b in range(KB)
                               if (b, t) in gi["tb"]]
                        nmm = sum(e[1][1] for e in tbs)
                        assert nmm > 0
                        pa = psa.tile([P, D], _F32, tag="pa",
                                      name=f"pa{L}_{g}_{t}")
                        done = 0
                        for b, (jb, njt, colg, so) in tbs:
                            for j in range(njt):
                                qt = qp.tile([P, P], _BF16, tag="q",
                                             name=f"q{L}_{g}_{t}_{b}_{j}")
                                build_q(qt, gi["col0"] + colg + j)
                                nc.tensor.matmul(
                                    pa[:], lhsT=qt[:],
                                    rhs=msg[:, jb + j, :D],
                                    start=(done == 0),
                                    stop=(done == nmm - 1))
                                done += 1
                        convert(pstage[:, tw * D:(tw + 1) * D], pa[:])
                    r0 = (g * GGT + wg * GT) * P
                    dst = partials[L][r0:r0 + GT * P, :].rearrange(
                        "(j p) c -> p j c", j=GT, p=P)
                    nc.sync.dma_start(dst, pstage[:].rearrange(
                        "p (j c) -> p j c", j=GT, c=D))

            def emit_phase_a(L):
                """Epilogue of layer L-1 (if L>0) fused with phase A of L."""
                rsall = None
                if L > 0:
                    rsall = rsp.tile([P, TILES * D], _BF16, tag="rs",
                                     name=f"rsall{L}")
                    src = rsouts[L - 1][:].rearrange(
                        "(j p) c -> p j c", j=TILES, p=P)
                    nc.sync.dma_start(rsall[:].rearrange(
                        "p (j c) -> p j c", j=TILES, c=D), src)
                for b in range(KB):
                    gstage = gsp.tile([P, HTILES * D], _BF16, tag="gs",
                                      name=f"gs{L}_{b}")
                    for th in range(HTILES):
                        t = b * HTILES + th
                        if L == 0:
                            lhs = ht0[:, t * P:(t + 1) * P]
                        else:
                            v = wk.tile([P, D], _BF16, tag="v",
                                        name=f"v{L}_{t}")
                            nc.scalar.mul(v[:], rsall[:, t * D:(t + 1) * D],
                                          dinv1[:, t:t + 1])
                            nc.vector.tensor_tensor(
                                out=v[:], in0=v[:],
                                in1=g2b[:, t * D:(t + 1) * D],
                                op=mybir.AluOpType.add)
                            h = wk.tile([P, D], _F32, tag="h",
                                        name=f"h{L}_{t}")
                            nc.scalar.activation(
                                h[:], v[:],
                                mybir.ActivationFunctionType.Lrelu,
                                bias=0.0, scale=1.0, alpha=NEG_SLOPE)
                            pt = pst.tile([D, P], _F32, tag="pt",
                                          name=f"pt{L}_{t}")
                            nc.tensor.transpose(pt[:], h[:], ident[:])
                            htp = htpp.tile([D, P], _BF16, tag="ht",
                                            name=f"htp{L}_{t}")
                            nc.scalar.copy(htp[:], pt[:])
                            lhs = htp[:]
                        pg = psg.tile([P, D], _F32, tag="pg",
                                      name=f"pg{L}_{t}")
                        nc.tensor.matmul(pg[:], lhsT=lhs, rhs=ws[L][:],
                                         start=True, stop=True)
                        nc.vector.tensor_scalar_mul(
                            gstage[:, th * D:(th + 1) * D], pg[:],
                            dinv1[:, t:t + 1])
                        g2 = wk.tile([P, D], _F32, tag="g2",
                                     name=f"g2_{L}_{t}")
                        nc.gpsimd.tensor_scalar_mul(g2[:], pg[:],
                                                    dinv2[:, t:t + 1])
                        nc.vector.tensor_tensor(
                            out=g2b[:, t * D:(t + 1) * D], in0=g2[:],
                            in1=bs[L][:], op=mybir.AluOpType.add)
                    dst = gtabs[L][b][:, :D].rearrange(
                        "(j p) c -> p j c", j=HTILES, p=P)
                    nc.sync.dma_start(dst, gstage[:].rearrange(
                        "p (j c) -> p j c", j=HTILES, c=D))

            for L in range(cfg.LAYERS):
                emit_phase_a(L)
                msgs = {0: emit_gathers(L, 0)}
                if NGG > 1:
                    msgs[1] = emit_gathers(L, 1)
                for g in range(NGG):
                    if g + 2 < NGG:
                        msgs[g + 2] = emit_gathers(L, g + 2)
                    emit_process(L, g, msgs.pop(g))
                nc.gpsimd.collective_compute(
                    "ReduceScatter",
                    mybir.AluOpType.add,
                    replica_groups=[list(range(CORES))],
                    ins=[partials[L][:]],
                    outs=[rsouts[L][:]],
                )

            # ---------- final epilogue -> output ----------
            L = cfg.LAYERS
            rsall = rsp.tile([P, TILES * D], _BF16, tag="rs", name="rsall_f")
            src = rsouts[L - 1][:].rearrange("(j p) c -> p j c", j=TILES, p=P)
            nc.sync.dma_start(rsall[:].rearrange(
                "p (j c) -> p j c", j=TILES, c=D), src)
            for t in range(TILES):
                v = wk.tile([P, D], _BF16, tag="v", name=f"vf_{t}")
                nc.scalar.mul(v[:], rsall[:, t * D:(t + 1) * D],
                              dinv1[:, t:t + 1])
                nc.vector.tensor_tensor(
                    out=v[:], in0=v[:],
                    in1=g2b[:, t * D:(t + 1) * D],
                    op=mybir.AluOpType.add)
                nc.scalar.activation(
                    hstage[:, t * D:(t + 1) * D], v[:],
                    mybir.ActivationFunctionType.Lrelu,
                    bias=0.0, scale=1.0, alpha=NEG_SLOPE)
            dst = out_t[:].rearrange("(j p) c -> p j c", j=TILES, p=P)
            nc.sync.dma_start(dst, hstage[:].rearrange(
                "p (j c) -> p j c", j=TILES, c=D))

    nc.compile()
    return nc


def make_in_maps(x, Ws, bss, meta, per_core, cfg):
    dinv = meta["dinv"]
    CORES, NPC, RPC, TILES = cfg.CORES, cfg.NPC, cfg.RPC, cfg.TILES
    import ml_dtypes
    iota_np = np.broadcast_to(np.arange(P).astype(ml_dtypes.bfloat16),
                              (P, P)).copy()
    in_maps = []
    for c in range(CORES):
        sl = slice(c * NPC, (c + 1) * NPC)
        xT = np.zeros((D, RPC), np.float32)
        xT[:, :NPC] = x[sl].T
        d1c = np.zeros(RPC, np.float32)
        d1c[:NPC] = dinv[sl]
        d1 = d1c.reshape(TILES, P).T.copy()
        d2 = (d1 * d1).astype(np.float32)
        im = {
            "xT": xT.astype(ml_dtypes.bfloat16),
            "dinv1": d1,
            "dinv2": d2,
            "iota": iota_np,
            "idx16": per_core[c]["idx16"],
            "dstrel": per_core[c]["dstrel"],
        }
        for i in range(3):
            im[f"W{i + 1}"] = Ws[i]
            im[f"bias{i + 1}"] = np.broadcast_to(
                bss[i], (P, D)).astype(np.float32).copy()
        in_maps.append(im)
    return in_maps


_CACHE = {}


def kernel(x, edge_index, W1, b1, W2, b2, W3, b3):
    cfg = DEFAULT_CFG
    x = np.asarray(x, dtype=np.float32)
    Ws = [np.asarray(w, dtype=np.float32) for w in (W1, W2, W3)]
    bss = [np.asarray(b, dtype=np.float32) for b in (b1, b2, b3)]

    ei = np.asarray(edge_index)
    key = hash(ei[:, ::997].tobytes()) ^ hash(ei.shape)
    if key not in _CACHE:
        meta, per_core = _preprocess(ei, cfg)
        nc = _build_program(meta, cfg)
        _CACHE[key] = (meta, per_core, nc)
    meta, per_core, nc = _CACHE[key]

    in_maps = make_in_maps(x, Ws, bss, meta, per_core, cfg)
    res = run_bass_kernel_spmd(nc, in_maps, core_ids=list(range(cfg.CORES)))
    out = np.empty((cfg.N, D), np.float32)
    for c in range(cfg.CORES):
        out[c * cfg.NPC:(c + 1) * cfg.NPC] = res.results[c]["out"][:cfg.NPC]
    return out


# revision 9
# speedup vs baseline: 1.1611x; 1.0239x over previous
"""3-layer GCN (GCNConv x3 + LeakyReLU, PyG semantics) on 8 Trainium2 cores.

Strategy (source-partitioned, ReduceScatter):
  - Core c owns nodes [c*NPC, (c+1)*NPC) and processes the edges whose SOURCE
    it owns.  Self loops are added as explicit edges, so a whole layer is
        OUT[d] = lrelu(dinv[d] * sum_{e: dst=d} G[src_e] + bias),
    with G = dinv * (H @ W) computed locally per core (no feature exchange).
  - Per layer: phase A computes G for own nodes into two local DRAM gather
    tables (low/high half of own rows, int16-indexable, 256B rows), then
    dma_gather over globally dst-sorted edges + one-hot matmul segment-sum
    into PSUM per global dst tile (784 tiles across all cores), converted to
    bf16 and written into a row-major [100352, 64] partial-sum table.
  - One ReduceScatter(add) delivers each core the full sum for its own
    12544 rows -- an output-sized collective (~57us) instead of AllGathering
    the whole feature table (~284us).
  - Epilogue per own tile: scale by dinv (ACT), add bias (DVE), LeakyReLU
    (ACT), PE transpose, and the next layer's phase-A matmul (bf16).
  - One-hot q matrices (is_equal(iota, drel)) are split across DVE and Pool;
    PSUM->bf16 converts across ACT/DVE, keeping every engine below the DMA
    roofline.  Gathers use one large call per (gather-group, table-half)
    span to amortize the ~1us SWDGE fixed cost per call.

The Bass program is SPMD: one program, per-core data.  Section lengths are
shared across cores (max over cores, padded with dummy edges whose one-hot
column is zero: dstrel = -1).
"""
import sys

sys.path.insert(0, "/opt/trn_rl_repo")

import numpy as np

import concourse.bacc as bacc
import concourse.mybir as mybir
import concourse.tile as tile
from concourse import library_config
from concourse.bass_utils import run_bass_kernel_spmd
from concourse.masks import make_identity

_F32 = mybir.dt.float32
_BF16 = mybir.dt.bfloat16
_I16 = mybir.dt.int16
P = 128
D = 64
NEG_SLOPE = 0.01


class Cfg:
    def __init__(self, n_nodes=100000, cores=8):
        self.N = n_nodes
        self.CORES = cores
        self.NPC = self.N // cores            # nodes owned per core
        self.TILES = (self.NPC + P - 1) // P  # own-node tiles per core (98)
        self.RPC = self.TILES * P             # padded rows per core (12544)
        self.GR = cores * self.RPC            # global padded rows (100352)
        self.GTILES = cores * self.TILES      # global dst tiles (784)
        self.KB = 1                           # source buckets (gather tables)
        assert self.RPC % self.KB == 0
        self.HALF = self.RPC // self.KB       # rows per gather table (12544)
        assert self.HALF <= 32767
        self.GGT = 28                         # dst tiles per gather group
        assert self.GTILES % self.GGT == 0
        self.NGG = self.GTILES // self.GGT    # gather groups (28)
        self.GT = 7                           # dst tiles per write group
        assert self.GGT % self.GT == 0
        self.WPG = self.GGT // self.GT        # write groups per gather group
        self.HTILES = self.TILES // self.KB   # own tiles per table (98)
        assert self.TILES % self.KB == 0
        self.LAYERS = 3
        self.MSGBUFS = 3
        self.QBUFS = 12
        self.PSABUFS = 4
        # engine split knobs
        self.Q_SPLIT = 3      # every Q_SPLIT-th q op goes to Pool
        self.CONV_MOD = 4     # converts: i%MOD==0 -> DVE, else ACT


DEFAULT_CFG = Cfg()


def _preprocess(edge_index, cfg):
    """Sort/pack edges; build per-core device arrays and shared metadata."""
    src0 = np.asarray(edge_index[0], dtype=np.int64)
    dst0 = np.asarray(edge_index[1], dtype=np.int64)
    N, CORES, NPC, TILES = cfg.N, cfg.CORES, cfg.NPC, cfg.TILES
    NGG, KB, GGT, HALF = cfg.NGG, cfg.KB, cfg.GGT, cfg.HALF

    src = src0
    dst = dst0

    deg = np.bincount(dst, minlength=N).astype(np.float32) + 1.0
    dinv = (1.0 / np.sqrt(deg)).astype(np.float32)

    owner = src // NPC                      # processing core (source owner)
    slocal = src % NPC
    bucket = slocal // HALF                 # gather-table half
    lrow = (slocal % HALF).astype(np.int16)

    downer = dst // NPC
    dloc = dst % NPC
    gtile = downer * TILES + dloc // P      # global dst tile 0..783
    drel_v = (dloc % P).astype(np.float32)
    grp = gtile // GGT
    tl = gtile % GGT

    counts = np.zeros((CORES, NGG, KB, GGT), dtype=np.int64)
    np.add.at(counts, (owner, grp, bucket, tl), 1)
    order = np.lexsort((tl, bucket, grp, owner))
    s_lrow = lrow[order]
    s_drel = drel_v[order]

    sec_len = counts.max(axis=0)            # [NGG, KB, GGT] shared sections

    # layout: gather group -> span (padded to x128) -> tile sections.
    # Each 128-edge block gets one drel column per covered tile PAIR: edges
    # of the pair's second tile store dstrel+128 and the one-hot is built
    # with a single 256-wide is_equal against iota256.
    ginfos = []
    tot_blocks = 0
    tot_cols = 0
    for g in range(NGG):
        gi = {"blk0": tot_blocks, "col0": tot_cols, "spans": {}, "tb": {},
              "bcols": [], "tmm": {}}
        assert KB == 1
        b = 0
        span_len = int(sec_len[g, b].sum())
        kgb = -(-span_len // P)
        gi["spans"][b] = (0, kgb)
        so = 0
        sec_rng = []                    # (t, start, end) in span coords
        for t in range(GGT):
            stb = int(sec_len[g, b, t])
            if stb == 0:
                continue
            gi["tb"][(b, t)] = (0, 0, 0, so)
            sec_rng.append((t, so, so + stb))
            so += stb
        # per block: covered tiles -> pair columns
        gcol = 0
        bcols = []                      # per block: (colbase, t0, ntiles)
        for j in range(kgb):
            lo, hi = j * P, (j + 1) * P
            tl_cov = [t for (t, a, e) in sec_rng if a < hi and e > lo]
            if not tl_cov:
                bcols.append((gcol, -1, 0))
                continue
            t0 = tl_cov[0]
            nt = len(tl_cov)
            assert tl_cov == list(range(t0, t0 + nt))
            bcols.append((gcol, t0, nt))
            gcol += -(-nt // 2)
        gi["bcols"] = bcols
        # per tile: ordered (block, pair m, side) matmul list
        for (t, a, e) in sec_rng:
            lst = []
            for j in range(a // P, (e - 1) // P + 1):
                colbase, t0, nt = bcols[j]
                if t0 < 0:
                    continue
                m = (t - t0) // 2
                side = (t - t0) % 2
                lst.append((j, m, side))
            gi["tmm"][t] = lst
        gi["blocks"] = kgb
        gi["ncols"] = gcol
        tot_blocks += kgb
        tot_cols += gcol
        ginfos.append(gi)
    tot_idx = tot_blocks * P

    # per-core run starts in the sorted edge list ((c, g, b, t)-major order)
    flat = counts.reshape(-1)
    starts = np.zeros(flat.size, dtype=np.int64)
    starts[1:] = np.cumsum(flat)[:-1]
    starts = starts.reshape(CORES, NGG, KB, GGT)

    per_core = []
    for c in range(CORES):
        lidx_flat = np.zeros(tot_idx, dtype=np.int16)
        drel = np.full((P, tot_cols), -1.0, dtype=np.float32)
        for g in range(NGG):
            gi = ginfos[g]
            bcol_base = np.array([bc[0] for bc in gi["bcols"]], dtype=np.int64)
            bcol_t0 = np.array([bc[1] for bc in gi["bcols"]], dtype=np.int64)
            for b, (bo, kgb) in gi["spans"].items():
                span_i0 = (gi["blk0"] + bo) * P
                for t in range(GGT):
                    if (b, t) not in gi["tb"]:
                        continue
                    so = gi["tb"][(b, t)][3]
                    n = int(counts[c, g, b, t])
                    if n == 0:
                        continue
                    s0 = int(starts[c, g, b, t])
                    pos0 = span_i0 + so
                    lidx_flat[pos0:pos0 + n] = s_lrow[s0:s0 + n]
                    q = so + np.arange(n)
                    j = q // P
                    pp = (pos0 + np.arange(n)) % P
                    toff = t - bcol_t0[j]
                    cols = gi["col0"] + bcol_base[j] + toff // 2
                    drel[pp, cols] = s_drel[s0:s0 + n] + 128.0 * (toff % 2)
        idx16 = np.tile(lidx_flat.reshape(tot_idx // 16, 16).T, (8, 1)).copy()
        per_core.append({"idx16": idx16, "dstrel": drel})

    meta = {
        "ginfos": ginfos,
        "tot_idx": tot_idx,
        "tot_cols": tot_cols,
        "dinv": dinv,
    }
    return meta, per_core


def _build_program(meta, cfg):
    ginfos = meta["ginfos"]
    tot_idx = meta["tot_idx"]
    tot_cols = meta["tot_cols"]
    CORES, TILES, RPC = cfg.CORES, cfg.TILES, cfg.RPC
    NGG, KB, GGT, GT = cfg.NGG, cfg.KB, cfg.GGT, cfg.GT
    HALF, HTILES, WPG, GR = cfg.HALF, cfg.HTILES, cfg.WPG, cfg.GR

    kmax_g = max(gi["blocks"] for gi in ginfos)

    nc = bacc.Bacc("TRN2", debug=False)
    nc.num_devices = CORES

    xT_in = nc.dram_tensor("xT", [D, RPC], _BF16, kind="ExternalInput")
    dinv1_in = nc.dram_tensor("dinv1", [P, TILES], _F32, kind="ExternalInput")
    dinv2_in = nc.dram_tensor("dinv2", [P, TILES], _F32, kind="ExternalInput")
    w_in = [nc.dram_tensor(f"W{i + 1}", [D, D], _F32, kind="ExternalInput")
            for i in range(3)]
    bias_in = [nc.dram_tensor(f"bias{i + 1}", [P, D], _F32,
                              kind="ExternalInput") for i in range(3)]
    iota_in = nc.dram_tensor("iota", [P, 2 * P], _BF16,
                            kind="ExternalInput")
    idx_in = nc.dram_tensor("idx16", [P, tot_idx // 16], _I16,
                            kind="ExternalInput")
    drel_in = nc.dram_tensor("dstrel", [P, tot_cols], _F32,
                             kind="ExternalInput")
    out_t = nc.dram_tensor("out", [RPC, D], _F32, kind="ExternalOutput")

    with tile.TileContext(nc) as tc:
        with tc.tile_pool(name="dram", bufs=1, space="DRAM") as dram, \
             tc.tile_pool(name="const", bufs=1) as cst, \
             tc.tile_pool(name="msgp", bufs=cfg.MSGBUFS) as msgp, \
             tc.tile_pool(name="qp", bufs=cfg.QBUFS) as qp, \
             tc.tile_pool(name="gsp", bufs=2) as gsp, \
             tc.tile_pool(name="psp", bufs=3) as psp, \
             tc.tile_pool(name="rsp", bufs=1) as rsp, \
             tc.tile_pool(name="wk", bufs=4) as wk, \
             tc.tile_pool(name="htp", bufs=4) as htpp, \
             tc.tile_pool(name="psa", bufs=cfg.PSABUFS, space="PSUM") as psa, \
             tc.tile_pool(name="psg", bufs=2, space="PSUM") as psg, \
             tc.tile_pool(name="pst", bufs=2, space="PSUM") as pst:

            nc.gpsimd.load_library(library_config.mlp)

            gtabs = [[dram.tile([HALF, 2 * D], _BF16, name=f"gt{L}_{b}")
                      for b in range(KB)] for L in range(cfg.LAYERS)]
            partials = [dram.tile([GR, D], _BF16, name=f"part{L}")
                        for L in range(cfg.LAYERS)]
            rsouts = [dram.tile([RPC, D], _BF16, name=f"rsout{L}")
                      for L in range(cfg.LAYERS)]

            iota = cst.tile([P, 2 * P], _BF16)
            nc.sync.dma_start(iota[:], iota_in[:])
            ident = cst.tile([P, P], _F32)
            make_identity(nc, ident[:])
            dinv1 = cst.tile([P, TILES], _F32)
            nc.sync.dma_start(dinv1[:], dinv1_in[:])
            dinv2 = cst.tile([P, TILES], _F32)
            nc.sync.dma_start(dinv2[:], dinv2_in[:])
            g2b = cst.tile([P, TILES * D], _BF16)
            ws, bs = [], []
            for i in range(3):
                w = cst.tile([D, D], _F32, name=f"w{i}")
                nc.sync.dma_start(w[:], w_in[i][:])
                wb = cst.tile([D, D], _BF16, name=f"wb{i}")
                nc.vector.tensor_copy(wb[:], w[:])
                ws.append(wb)
                bt = cst.tile([P, D], _F32, name=f"b{i}")
                nc.sync.dma_start(bt[:], bias_in[i][:])
                bs.append(bt)
            ht0 = cst.tile([D, RPC], _BF16)        # layer-0 input (x.T)
            nc.sync.dma_start(ht0[:], xT_in[:])

            # zero upper halves of the gather tables (once)
            zstage = cst.tile([P, HTILES * D], _BF16)
            nc.gpsimd.memset(zstage[:], 0.0)
            for L in range(cfg.LAYERS):
                for b in range(KB):
                    dst = gtabs[L][b][:, D:2 * D].rearrange(
                        "(j p) c -> p j c", j=HTILES, p=P)
                    nc.sync.dma_start(dst, zstage[:].rearrange(
                        "p (j c) -> p j c", j=HTILES, c=D))

            idx_sb = cst.tile([P, tot_idx // 16], _I16)
            nc.sync.dma_start(idx_sb[:], idx_in[:])
            drel_sb = cst.tile([P, tot_cols], _F32)
            nc.sync.dma_start(drel_sb[:], drel_in[:])

            hstage = cst.tile([P, TILES * D], _F32)

            qctr = [0]
            cctr = [0]

            def build_q(qt, col, width):
                # wide (256) one-hots go to DVE (2-elem/cycle); narrow ones
                # alternate DVE/Pool per Q_SPLIT
                if width > P:
                    eng = nc.vector
                else:
                    eng = (nc.gpsimd
                           if (qctr[0] % cfg.Q_SPLIT == cfg.Q_SPLIT - 1)
                           else nc.vector)
                    qctr[0] += 1
                eng.tensor_scalar(
                    out=qt[:, :width], in0=iota[:, :width],
                    scalar1=drel_sb[:, col:col + 1], scalar2=None,
                    op0=mybir.AluOpType.is_equal)

            def convert(dst_ap, src_ap):
                m = cctr[0] % cfg.CONV_MOD
                cctr[0] += 1
                if m == 0:
                    nc.vector.tensor_copy(dst_ap, src_ap)
                else:
                    nc.scalar.copy(dst_ap, src_ap)

            def emit_gathers(L, g):
                gi = ginfos[g]
                msg = msgp.tile([P, gi["blocks"], 2 * D], _BF16,
                                tag="msg", name=f"msg{L}_{g}",
                                padded_shape=[P, kmax_g, 2 * D])
                for b, (bo, kgb) in gi["spans"].items():
                    i0 = (gi["blk0"] + bo) * P
                    nidx = kgb * P
                    nc.gpsimd.dma_gather(
                        msg[:, bo:bo + kgb, :],
                        gtabs[L][b][:],
                        idx_sb[:, i0 // 16:(i0 + nidx) // 16],
                        nidx, nidx, 2 * D,
                        single_packet=False)
                return msg

            def emit_process(L, g, msg):
                gi = ginfos[g]
                bcols = gi["bcols"]
                qtiles = {}                      # (j, m) -> q tile

                def get_q(j, m):
                    key = (j, m)
                    if key not in qtiles:
                        colbase, t0, nt = bcols[j]
                        width = 2 * P if (nt - 2 * m) >= 2 else P
                        qt = qp.tile([P, 2 * P], _BF16, tag="q",
                                     name=f"q{L}_{g}_{j}_{m}")
                        build_q(qt, gi["col0"] + colbase + m, width)
                        qtiles[key] = qt
                    return qtiles[key]

                for wg in range(WPG):
                    pstage = psp.tile([P, GT * D], _BF16, tag="ps",
                                      name=f"ps{L}_{g}_{wg}")
                    for tw in range(GT):
                        t = wg * GT + tw
                        tmm = gi["tmm"].get(t, [])
                        nmm = len(tmm)
                        assert nmm > 0
                        pa = psa.tile([P, D], _F32, tag="pa",
                                      name=f"pa{L}_{g}_{t}")
                        for done, (j, m, side) in enumerate(tmm):
                            qt = get_q(j, m)
                            nc.tensor.matmul(
                                pa[:], lhsT=qt[:, side * P:(side + 1) * P],
                                rhs=msg[:, j, :D],
                                start=(done == 0),
                                stop=(done == nmm - 1))
                        convert(pstage[:, tw * D:(tw + 1) * D], pa[:])
                    r0 = (g * GGT + wg * GT) * P
                    dst = partials[L][r0:r0 + GT * P, :].rearrange(
                        "(j p) c -> p j c", j=GT, p=P)
                    nc.sync.dma_start(dst, pstage[:].rearrange(
                        "p (j c) -> p j c", j=GT, c=D))

            def emit_phase_a(L):
                """Epilogue of layer L-1 (if L>0) fused with phase A of L."""
                rsall = None
                if L > 0:
                    rsall = rsp.tile([P, TILES * D], _BF16, tag="rs",
                                     name=f"rsall{L}")
                    src = rsouts[L - 1][:].rearrange(
                        "(j p) c -> p j c", j=TILES, p=P)
                    nc.sync.dma_start(rsall[:].rearrange(
                        "p (j c) -> p j c", j=TILES, c=D), src)
                for b in range(KB):
                    gstage = gsp.tile([P, HTILES * D], _BF16, tag="gs",
                                      name=f"gs{L}_{b}")
                    for th in range(HTILES):
                        t = b * HTILES + th
                        if L == 0:
                            lhs = ht0[:, t * P:(t + 1) * P]
                        else:
                            v = wk.tile([P, D], _BF16, tag="v",
                                        name=f"v{L}_{t}")
                            nc.scalar.mul(v[:], rsall[:, t * D:(t + 1) * D],
                                          dinv1[:, t:t + 1])
                            nc.vector.tensor_tensor(
                                out=v[:], in0=v[:],
                                in1=g2b[:, t * D:(t + 1) * D],
                                op=mybir.AluOpType.add)
                            h = wk.tile([P, D], _F32, tag="h",
                                        name=f"h{L}_{t}")
                            nc.scalar.activation(
                                h[:], v[:],
                                mybir.ActivationFunctionType.Lrelu,
                                bias=0.0, scale=1.0, alpha=NEG_SLOPE)
                            pt = pst.tile([D, P], _F32, tag="pt",
                                          name=f"pt{L}_{t}")
                            nc.tensor.transpose(pt[:], h[:], ident[:])
                            htp = htpp.tile([D, P], _BF16, tag="ht",
                                            name=f"htp{L}_{t}")
                            nc.vector.tensor_copy(htp[:], pt[:])
                            lhs = htp[:]
                        pg = psg.tile([P, D], _F32, tag="pg",
                                      name=f"pg{L}_{t}")
                        nc.tensor.matmul(pg[:], lhsT=lhs, rhs=ws[L][:],
                                         start=True, stop=True)
                        nc.vector.tensor_scalar_mul(
                            gstage[:, th * D:(th + 1) * D], pg[:],
                            dinv1[:, t:t + 1])
                        g2 = wk.tile([P, D], _F32, tag="g2",
                                     name=f"g2_{L}_{t}")
                        nc.gpsimd.tensor_scalar_mul(g2[:], pg[:],
                                                    dinv2[:, t:t + 1])
                        nc.vector.tensor_tensor(
                            out=g2b[:, t * D:(t + 1) * D], in0=g2[:],
                            in1=bs[L][:], op=mybir.AluOpType.add)
                    dst = gtabs[L][b][:, :D].rearrange(
                        "(j p) c -> p j c", j=HTILES, p=P)
                    nc.sync.dma_start(dst, gstage[:].rearrange(
                        "p (j c) -> p j c", j=HTILES, c=D))

            for L in range(cfg.LAYERS):
                emit_phase_a(L)
                msgs = {0: emit_gathers(L, 0)}
                if NGG > 1:
                    msgs[1] = emit_gathers(L, 1)
                for g in range(NGG):
                    if g + 2 < NGG:
                        msgs[g + 2] = emit_gathers(L, g + 2)
                    emit_process(L, g, msgs.pop(g))
                nc.gpsimd.collective_compute(
                    "ReduceScatter",
                    mybir.AluOpType.add,
                    replica_groups=[list(range(CORES))],
                    ins=[partials[L][:]],
                    outs=[rsouts[L][:]],
                )

            # ---------- final epilogue -> output ----------
            L = cfg.LAYERS
            rsall = rsp.tile([P, TILES * D], _BF16, tag="rs", name="rsall_f")
            src = rsouts[L - 1][:].rearrange("(j p) c -> p j c", j=TILES, p=P)
            nc.sync.dma_start(rsall[:].rearrange(
                "p (j c) -> p j c", j=TILES, c=D), src)
            for t in range(TILES):
                v = wk.tile([P, D], _BF16, tag="v", name=f"vf_{t}")
                nc.scalar.mul(v[:], rsall[:, t * D:(t + 1) * D],
                              dinv1[:, t:t + 1])
                nc.vector.tensor_tensor(
                    out=v[:], in0=v[:],
                    in1=g2b[:, t * D:(t + 1) * D],
                    op=mybir.AluOpType.add)
                nc.scalar.activation(
                    hstage[:, t * D:(t + 1) * D], v[:],
                    mybir.ActivationFunctionType.Lrelu,
                    bias=0.0, scale=1.0, alpha=NEG_SLOPE)
            dst = out_t[:].rearrange("(j p) c -> p j c", j=TILES, p=P)
            nc.sync.dma_start(dst, hstage[:].rearrange(
                "p (j c) -> p j c", j=TILES, c=D))

    nc.compile()
    return nc


def make_in_maps(x, Ws, bss, meta, per_core, cfg):
    dinv = meta["dinv"]
    CORES, NPC, RPC, TILES = cfg.CORES, cfg.NPC, cfg.RPC, cfg.TILES
    import ml_dtypes
    iota_np = np.broadcast_to(np.arange(2 * P).astype(ml_dtypes.bfloat16),
                              (P, 2 * P)).copy()
    in_maps = []
    for c in range(CORES):
        sl = slice(c * NPC, (c + 1) * NPC)
        xT = np.zeros((D, RPC), np.float32)
        xT[:, :NPC] = x[sl].T
        d1c = np.zeros(RPC, np.float32)
        d1c[:NPC] = dinv[sl]
        d1 = d1c.reshape(TILES, P).T.copy()
        d2 = (d1 * d1).astype(np.float32)
        im = {
            "xT": xT.astype(ml_dtypes.bfloat16),
            "dinv1": d1,
            "dinv2": d2,
            "iota": iota_np,
            "idx16": per_core[c]["idx16"],
            "dstrel": per_core[c]["dstrel"],
        }
        for i in range(3):
            im[f"W{i + 1}"] = Ws[i]
            im[f"bias{i + 1}"] = np.broadcast_to(
                bss[i], (P, D)).astype(np.float32).copy()
        in_maps.append(im)
    return in_maps


_CACHE = {}


def kernel(x, edge_index, W1, b1, W2, b2, W3, b3):
    cfg = DEFAULT_CFG
    x = np.asarray(x, dtype=np.float32)
    Ws = [np.asarray(w, dtype=np.float32) for w in (W1, W2, W3)]
    bss = [np.asarray(b, dtype=np.float32) for b in (b1, b2, b3)]

    ei = np.asarray(edge_index)
    key = hash(ei[:, ::997].tobytes()) ^ hash(ei.shape)
    if key not in _CACHE:
        meta, per_core = _preprocess(ei, cfg)
        nc = _build_program(meta, cfg)
        _CACHE[key] = (meta, per_core, nc)
    meta, per_core, nc = _CACHE[key]

    in_maps = make_in_maps(x, Ws, bss, meta, per_core, cfg)
    res = run_bass_kernel_spmd(nc, in_maps, core_ids=list(range(cfg.CORES)))
    out = np.empty((cfg.N, D), np.float32)
    for c in range(cfg.CORES):
        out[c * cfg.NPC:(c + 1) * cfg.NPC] = res.results[c]["out"][:cfg.NPC]
    return out


# revision 12
# speedup vs baseline: 1.6195x; 1.3948x over previous
"""3-layer GCN (GCNConv x3 + LeakyReLU, PyG semantics) on 8 Trainium2 cores.

Strategy (source-partitioned, ReduceScatter):
  - Core c owns nodes [c*NPC, (c+1)*NPC) and processes the edges whose SOURCE
    it owns.  Self loops are added as explicit edges, so a whole layer is
        OUT[d] = lrelu(dinv[d] * sum_{e: dst=d} G[src_e] + bias),
    with G = dinv * (H @ W) computed locally per core (no feature exchange).
  - Per layer: phase A computes G for own nodes into two local DRAM gather
    tables (low/high half of own rows, int16-indexable, 256B rows), then
    dma_gather over globally dst-sorted edges + one-hot matmul segment-sum
    into PSUM per global dst tile (784 tiles across all cores), converted to
    bf16 and written into a row-major [100352, 64] partial-sum table.
  - One ReduceScatter(add) delivers each core the full sum for its own
    12544 rows -- an output-sized collective (~57us) instead of AllGathering
    the whole feature table (~284us).
  - Epilogue per own tile: scale by dinv (ACT), add bias (DVE), LeakyReLU
    (ACT), PE transpose, and the next layer's phase-A matmul (bf16).
  - One-hot q matrices (is_equal(iota, drel)) are split across DVE and Pool;
    PSUM->bf16 converts across ACT/DVE, keeping every engine below the DMA
    roofline.  Gathers use one large call per (gather-group, table-half)
    span to amortize the ~1us SWDGE fixed cost per call.

The Bass program is SPMD: one program, per-core data.  Section lengths are
shared across cores (max over cores, padded with dummy edges whose one-hot
column is zero: dstrel = -1).
"""
import sys

sys.path.insert(0, "/opt/trn_rl_repo")

import numpy as np

import concourse.bacc as bacc
import concourse.mybir as mybir
import concourse.tile as tile
from concourse import library_config
from concourse.bass_utils import run_bass_kernel_spmd
from concourse.masks import make_identity

_F32 = mybir.dt.float32
_BF16 = mybir.dt.bfloat16
_I16 = mybir.dt.int16
P = 128
D = 64
NEG_SLOPE = 0.01


class Cfg:
    def __init__(self, n_nodes=100000, cores=8):
        self.N = n_nodes
        self.CORES = cores
        self.NPC = self.N // cores            # nodes owned per core
        self.TILES = (self.NPC + P - 1) // P  # own-node tiles per core (98)
        self.RPC = self.TILES * P             # padded rows per core (12544)
        self.GR = cores * self.RPC            # global padded rows (100352)
        self.GTILES = cores * self.TILES      # global dst tiles (784)
        self.KB = 1                           # source buckets (gather tables)
        assert self.RPC % self.KB == 0
        self.HALF = self.RPC // self.KB       # rows per gather table (12544)
        assert self.HALF <= 32767
        self.GGT = 28                         # dst tiles per gather group
        assert self.GTILES % self.GGT == 0
        self.NGG = self.GTILES // self.GGT    # gather groups (28)
        self.GT = 7                           # dst tiles per write group
        assert self.GGT % self.GT == 0
        self.WPG = self.GGT // self.GT        # write groups per gather group
        self.HTILES = self.TILES // self.KB   # own tiles per table (98)
        assert self.TILES % self.KB == 0
        self.LAYERS = 3
        self.MSGBUFS = 3
        self.QBUFS = 12
        self.PSABUFS = 4
        # engine split knobs
        self.Q_SPLIT = 1000000      # every Q_SPLIT-th q op goes to Pool
        self.CONV_MOD = 1000000     # converts: i%MOD==0 -> DVE, else ACT


DEFAULT_CFG = Cfg()


def _preprocess(edge_index, cfg):
    """Sort/pack edges; build per-core device arrays and shared metadata."""
    src0 = np.asarray(edge_index[0], dtype=np.int64)
    dst0 = np.asarray(edge_index[1], dtype=np.int64)
    N, CORES, NPC, TILES = cfg.N, cfg.CORES, cfg.NPC, cfg.TILES
    NGG, KB, GGT, HALF = cfg.NGG, cfg.KB, cfg.GGT, cfg.HALF

    src = src0
    dst = dst0

    deg = np.bincount(dst, minlength=N).astype(np.float32) + 1.0
    dinv = (1.0 / np.sqrt(deg)).astype(np.float32)

    owner = src // NPC                      # processing core (source owner)
    slocal = src % NPC
    bucket = slocal // HALF                 # gather-table half
    lrow = (slocal % HALF).astype(np.int16)

    downer = dst // NPC
    dloc = dst % NPC
    gtile = downer * TILES + dloc // P      # global dst tile 0..783
    drel_v = (dloc % P).astype(np.float32)
    grp = gtile // GGT
    tl = gtile % GGT

    counts = np.zeros((CORES, NGG, KB, GGT), dtype=np.int64)
    np.add.at(counts, (owner, grp, bucket, tl), 1)
    order = np.lexsort((tl, bucket, grp, owner))
    s_lrow = lrow[order]
    s_drel = drel_v[order]

    sec_len = counts.max(axis=0)            # [NGG, KB, GGT] shared sections

    # layout: gather group -> span (padded to x128) -> tile sections.
    # Each 128-edge block gets one drel column per covered tile PAIR: edges
    # of the pair's second tile store dstrel+128 and the one-hot is built
    # with a single 256-wide is_equal against iota256.
    ginfos = []
    tot_blocks = 0
    tot_cols = 0
    for g in range(NGG):
        gi = {"blk0": tot_blocks, "col0": tot_cols, "spans": {}, "tb": {},
              "bcols": [], "tmm": {}}
        assert KB == 1
        b = 0
        span_len = int(sec_len[g, b].sum())
        kgb = -(-span_len // P)
        gi["spans"][b] = (0, kgb)
        so = 0
        sec_rng = []                    # (t, start, end) in span coords
        for t in range(GGT):
            stb = int(sec_len[g, b, t])
            if stb == 0:
                continue
            gi["tb"][(b, t)] = (0, 0, 0, so)
            sec_rng.append((t, so, so + stb))
            so += stb
        # per block: covered tiles -> pair columns
        gcol = 0
        bcols = []                      # per block: (colbase, t0, ntiles)
        for j in range(kgb):
            lo, hi = j * P, (j + 1) * P
            tl_cov = [t for (t, a, e) in sec_rng if a < hi and e > lo]
            if not tl_cov:
                bcols.append((gcol, -1, 0))
                continue
            t0 = tl_cov[0]
            nt = len(tl_cov)
            assert tl_cov == list(range(t0, t0 + nt))
            bcols.append((gcol, t0, nt))
            gcol += -(-nt // 2)
        gi["bcols"] = bcols
        # per tile: ordered (block, pair m, side) matmul list
        for (t, a, e) in sec_rng:
            lst = []
            for j in range(a // P, (e - 1) // P + 1):
                colbase, t0, nt = bcols[j]
                if t0 < 0:
                    continue
                m = (t - t0) // 2
                side = (t - t0) % 2
                lst.append((j, m, side))
            gi["tmm"][t] = lst
        gi["blocks"] = kgb
        gi["ncols"] = gcol
        tot_blocks += kgb
        tot_cols += gcol
        ginfos.append(gi)
    tot_idx = tot_blocks * P

    # per-core run starts in the sorted edge list ((c, g, b, t)-major order)
    flat = counts.reshape(-1)
    starts = np.zeros(flat.size, dtype=np.int64)
    starts[1:] = np.cumsum(flat)[:-1]
    starts = starts.reshape(CORES, NGG, KB, GGT)

    per_core = []
    for c in range(CORES):
        lidx_flat = np.zeros(tot_idx, dtype=np.int16)
        drel = np.full((P, tot_cols), -1.0, dtype=np.float32)
        for g in range(NGG):
            gi = ginfos[g]
            bcol_base = np.array([bc[0] for bc in gi["bcols"]], dtype=np.int64)
            bcol_t0 = np.array([bc[1] for bc in gi["bcols"]], dtype=np.int64)
            for b, (bo, kgb) in gi["spans"].items():
                span_i0 = (gi["blk0"] + bo) * P
                for t in range(GGT):
                    if (b, t) not in gi["tb"]:
                        continue
                    so = gi["tb"][(b, t)][3]
                    n = int(counts[c, g, b, t])
                    if n == 0:
                        continue
                    s0 = int(starts[c, g, b, t])
                    pos0 = span_i0 + so
                    lidx_flat[pos0:pos0 + n] = s_lrow[s0:s0 + n]
                    q = so + np.arange(n)
                    j = q // P
                    pp = (pos0 + np.arange(n)) % P
                    toff = t - bcol_t0[j]
                    cols = gi["col0"] + bcol_base[j] + toff // 2
                    drel[pp, cols] = s_drel[s0:s0 + n] + 128.0 * (toff % 2)
        idx16 = np.tile(lidx_flat.reshape(tot_idx // 16, 16).T, (8, 1)).copy()
        per_core.append({"idx16": idx16, "dstrel": drel})

    meta = {
        "ginfos": ginfos,
        "tot_idx": tot_idx,
        "tot_cols": tot_cols,
        "dinv": dinv,
    }
    return meta, per_core


def _build_program(meta, cfg):
    ginfos = meta["ginfos"]
    tot_idx = meta["tot_idx"]
    tot_cols = meta["tot_cols"]
    CORES, TILES, RPC = cfg.CORES, cfg.TILES, cfg.RPC
    NGG, KB, GGT, GT = cfg.NGG, cfg.KB, cfg.GGT, cfg.GT
    HALF, HTILES, WPG, GR = cfg.HALF, cfg.HTILES, cfg.WPG, cfg.GR

    kmax_g = max(gi["blocks"] for gi in ginfos)

    nc = bacc.Bacc("TRN2", debug=False)
    nc.num_devices = CORES

    xT_in = nc.dram_tensor("xT", [D, RPC], _BF16, kind="ExternalInput")
    dinv1_in = nc.dram_tensor("dinv1", [P, TILES], _F32, kind="ExternalInput")
    dinv2_in = nc.dram_tensor("dinv2", [P, TILES], _F32, kind="ExternalInput")
    w_in = [nc.dram_tensor(f"W{i + 1}", [D, D], _F32, kind="ExternalInput")
            for i in range(3)]
    bias_in = [nc.dram_tensor(f"bias{i + 1}", [P, D], _F32,
                              kind="ExternalInput") for i in range(3)]
    iota_in = nc.dram_tensor("iota", [P, 2 * P], _BF16,
                            kind="ExternalInput")
    idx_in = nc.dram_tensor("idx16", [P, tot_idx // 16], _I16,
                            kind="ExternalInput")
    drel_in = nc.dram_tensor("dstrel", [P, tot_cols], _F32,
                             kind="ExternalInput")
    out_t = nc.dram_tensor("out", [RPC, D], _F32, kind="ExternalOutput")

    with tile.TileContext(nc) as tc:
        with tc.tile_pool(name="dram", bufs=1, space="DRAM") as dram, \
             tc.tile_pool(name="const", bufs=1) as cst, \
             tc.tile_pool(name="msgp", bufs=cfg.MSGBUFS) as msgp, \
             tc.tile_pool(name="qp", bufs=cfg.QBUFS) as qp, \
             tc.tile_pool(name="gsp", bufs=2) as gsp, \
             tc.tile_pool(name="psp", bufs=3) as psp, \
             tc.tile_pool(name="rsp", bufs=1) as rsp, \
             tc.tile_pool(name="wk", bufs=4) as wk, \
             tc.tile_pool(name="htp", bufs=4) as htpp, \
             tc.tile_pool(name="psa", bufs=cfg.PSABUFS, space="PSUM") as psa, \
             tc.tile_pool(name="psg", bufs=2, space="PSUM") as psg, \
             tc.tile_pool(name="pst", bufs=2, space="PSUM") as pst:

            nc.gpsimd.load_library(library_config.mlp)

            gtabs = [[dram.tile([HALF, 2 * D], _BF16, name=f"gt{L}_{b}")
                      for b in range(KB)] for L in range(cfg.LAYERS)]
            partials = [dram.tile([GR, D], _BF16, name=f"part{L}")
                        for L in range(cfg.LAYERS)]
            rsouts = [dram.tile([RPC, D], _BF16, name=f"rsout{L}")
                      for L in range(cfg.LAYERS)]

            iota = cst.tile([P, 2 * P], _BF16)
            nc.sync.dma_start(iota[:], iota_in[:])
            ident = cst.tile([P, P], _F32)
            make_identity(nc, ident[:])
            dinv1 = cst.tile([P, TILES], _F32)
            nc.sync.dma_start(dinv1[:], dinv1_in[:])
            dinv2 = cst.tile([P, TILES], _F32)
            nc.sync.dma_start(dinv2[:], dinv2_in[:])
            g2b = cst.tile([P, TILES * D], _BF16)
            ws, bs = [], []
            for i in range(3):
                w = cst.tile([D, D], _F32, name=f"w{i}")
                nc.sync.dma_start(w[:], w_in[i][:])
                wb = cst.tile([D, D], _BF16, name=f"wb{i}")
                nc.vector.tensor_copy(wb[:], w[:])
                ws.append(wb)
                bt = cst.tile([P, D], _F32, name=f"b{i}")
                nc.sync.dma_start(bt[:], bias_in[i][:])
                bs.append(bt)
            ht0 = cst.tile([D, RPC], _BF16)        # layer-0 input (x.T)
            nc.sync.dma_start(ht0[:], xT_in[:])

            # zero upper halves of the gather tables (once)
            zstage = cst.tile([P, HTILES * D], _BF16)
            nc.gpsimd.memset(zstage[:], 0.0)
            for L in range(cfg.LAYERS):
                for b in range(KB):
                    dst = gtabs[L][b][:, D:2 * D].rearrange(
                        "(j p) c -> p j c", j=HTILES, p=P)
                    nc.sync.dma_start(dst, zstage[:].rearrange(
                        "p (j c) -> p j c", j=HTILES, c=D))

            idx_sb = cst.tile([P, tot_idx // 16], _I16)
            nc.sync.dma_start(idx_sb[:], idx_in[:])
            drel_sb = cst.tile([P, tot_cols], _F32)
            nc.sync.dma_start(drel_sb[:], drel_in[:])

            hstage = cst.tile([P, TILES * D], _F32)

            qctr = [0]
            cctr = [0]

            def build_q(qt, col, width):
                # wide (256) one-hots go to DVE (2-elem/cycle); narrow ones
                # alternate DVE/Pool per Q_SPLIT
                if width > P:
                    eng = nc.vector
                else:
                    eng = (nc.gpsimd
                           if (qctr[0] % cfg.Q_SPLIT == cfg.Q_SPLIT - 1)
                           else nc.vector)
                    qctr[0] += 1
                eng.tensor_scalar(
                    out=qt[:, :width], in0=iota[:, :width],
                    scalar1=drel_sb[:, col:col + 1], scalar2=None,
                    op0=mybir.AluOpType.is_equal)

            def convert(dst_ap, src_ap):
                m = cctr[0] % cfg.CONV_MOD
                cctr[0] += 1
                if m == 0:
                    nc.vector.tensor_copy(dst_ap, src_ap)
                else:
                    nc.scalar.copy(dst_ap, src_ap)

            def emit_gathers(L, g):
                gi = ginfos[g]
                msg = msgp.tile([P, gi["blocks"], 2 * D], _BF16,
                                tag="msg", name=f"msg{L}_{g}",
                                padded_shape=[P, kmax_g, 2 * D])
                for b, (bo, kgb) in gi["spans"].items():
                    i0 = (gi["blk0"] + bo) * P
                    nidx = kgb * P
                    nc.gpsimd.dma_gather(
                        msg[:, bo:bo + kgb, :],
                        gtabs[L][b][:],
                        idx_sb[:, i0 // 16:(i0 + nidx) // 16],
                        nidx, nidx, 2 * D,
                        single_packet=False)
                return msg

            def emit_process(L, g, msg):
                gi = ginfos[g]
                bcols = gi["bcols"]
                qtiles = {}                      # (j, m) -> q tile

                def get_q(j, m):
                    key = (j, m)
                    if key not in qtiles:
                        colbase, t0, nt = bcols[j]
                        width = 2 * P if (nt - 2 * m) >= 2 else P
                        qt = qp.tile([P, 2 * P], _BF16, tag="q",
                                     name=f"q{L}_{g}_{j}_{m}")
                        build_q(qt, gi["col0"] + colbase + m, width)
                        qtiles[key] = qt
                    return qtiles[key]

                for wg in range(WPG):
                    pstage = psp.tile([P, GT * D], _BF16, tag="ps",
                                      name=f"ps{L}_{g}_{wg}")
                    for tw in range(GT):
                        t = wg * GT + tw
                        tmm = gi["tmm"].get(t, [])
                        nmm = len(tmm)
                        assert nmm > 0
                        pa = psa.tile([P, D], _F32, tag="pa",
                                      name=f"pa{L}_{g}_{t}")
                        for done, (j, m, side) in enumerate(tmm):
                            qt = get_q(j, m)
                            nc.tensor.matmul(
                                pa[:], lhsT=qt[:, side * P:(side + 1) * P],
                                rhs=msg[:, j, :D],
                                start=(done == 0),
                                stop=(done == nmm - 1))
                        convert(pstage[:, tw * D:(tw + 1) * D], pa[:])
                    r0 = (g * GGT + wg * GT) * P
                    dst = partials[L][r0:r0 + GT * P, :].rearrange(
                        "(j p) c -> p j c", j=GT, p=P)
                    nc.sync.dma_start(dst, pstage[:].rearrange(
                        "p (j c) -> p j c", j=GT, c=D))

            def emit_phase_a(L):
                """Epilogue of layer L-1 (if L>0) fused with phase A of L."""
                rsall = None
                if L > 0:
                    rsall = rsp.tile([P, TILES * D], _BF16, tag="rs",
                                     name=f"rsall{L}")
                    src = rsouts[L - 1][:].rearrange(
                        "(j p) c -> p j c", j=TILES, p=P)
                    nc.sync.dma_start(rsall[:].rearrange(
                        "p (j c) -> p j c", j=TILES, c=D), src)
                for b in range(KB):
                    gstage = gsp.tile([P, HTILES * D], _BF16, tag="gs",
                                      name=f"gs{L}_{b}")
                    for th in range(HTILES):
                        t = b * HTILES + th
                        if L == 0:
                            lhs = ht0[:, t * P:(t + 1) * P]
                        else:
                            v = wk.tile([P, D], _BF16, tag="v",
                                        name=f"v{L}_{t}")
                            nc.scalar.mul(v[:], rsall[:, t * D:(t + 1) * D],
                                          dinv1[:, t:t + 1])
                            nc.vector.tensor_tensor(
                                out=v[:], in0=v[:],
                                in1=g2b[:, t * D:(t + 1) * D],
                                op=mybir.AluOpType.add)
                            h = wk.tile([P, D], _F32, tag="h",
                                        name=f"h{L}_{t}")
                            nc.scalar.activation(
                                h[:], v[:],
                                mybir.ActivationFunctionType.Lrelu,
                                bias=0.0, scale=1.0, alpha=NEG_SLOPE)
                            pt = pst.tile([D, P], _F32, tag="pt",
                                          name=f"pt{L}_{t}")
                            nc.tensor.transpose(pt[:], h[:], ident[:])
                            htp = htpp.tile([D, P], _BF16, tag="ht",
                                            name=f"htp{L}_{t}")
                            nc.vector.tensor_copy(htp[:], pt[:])
                            lhs = htp[:]
                        pg = psg.tile([P, D], _F32, tag="pg",
                                      name=f"pg{L}_{t}")
                        nc.tensor.matmul(pg[:], lhsT=lhs, rhs=ws[L][:],
                                         start=True, stop=True)
                        nc.vector.tensor_scalar_mul(
                            gstage[:, th * D:(th + 1) * D], pg[:],
                            dinv1[:, t:t + 1])
                        g2 = wk.tile([P, D], _F32, tag="g2",
                                     name=f"g2_{L}_{t}")
                        nc.vector.tensor_scalar_mul(g2[:], pg[:],
                                                    dinv2[:, t:t + 1])
                        nc.vector.tensor_tensor(
                            out=g2b[:, t * D:(t + 1) * D], in0=g2[:],
                            in1=bs[L][:], op=mybir.AluOpType.add)
                    dst = gtabs[L][b][:, :D].rearrange(
                        "(j p) c -> p j c", j=HTILES, p=P)
                    nc.sync.dma_start(dst, gstage[:].rearrange(
                        "p (j c) -> p j c", j=HTILES, c=D))

            for L in range(cfg.LAYERS):
                emit_phase_a(L)
                msgs = {0: emit_gathers(L, 0)}
                if NGG > 1:
                    msgs[1] = emit_gathers(L, 1)
                for g in range(NGG):
                    if g + 2 < NGG:
                        msgs[g + 2] = emit_gathers(L, g + 2)
                    emit_process(L, g, msgs.pop(g))
                nc.gpsimd.collective_compute(
                    "ReduceScatter",
                    mybir.AluOpType.add,
                    replica_groups=[list(range(CORES))],
                    ins=[partials[L][:]],
                    outs=[rsouts[L][:]],
                )

            # ---------- final epilogue -> output ----------
            L = cfg.LAYERS
            rsall = rsp.tile([P, TILES * D], _BF16, tag="rs", name="rsall_f")
            src = rsouts[L - 1][:].rearrange("(j p) c -> p j c", j=TILES, p=P)
            nc.sync.dma_start(rsall[:].rearrange(
                "p (j c) -> p j c", j=TILES, c=D), src)
            for t in range(TILES):
                v = wk.tile([P, D], _BF16, tag="v", name=f"vf_{t}")
                nc.scalar.mul(v[:], rsall[:, t * D:(t + 1) * D],
                              dinv1[:, t:t + 1])
                nc.vector.tensor_tensor(
                    out=v[:], in0=v[:],
                    in1=g2b[:, t * D:(t + 1) * D],
                    op=mybir.AluOpType.add)
                nc.scalar.activation(
                    hstage[:, t * D:(t + 1) * D], v[:],
                    mybir.ActivationFunctionType.Lrelu,
                    bias=0.0, scale=1.0, alpha=NEG_SLOPE)
            dst = out_t[:].rearrange("(j p) c -> p j c", j=TILES, p=P)
            nc.sync.dma_start(dst, hstage[:].rearrange(
                "p (j c) -> p j c", j=TILES, c=D))

    nc.compile()
    return nc


def make_in_maps(x, Ws, bss, meta, per_core, cfg):
    dinv = meta["dinv"]
    CORES, NPC, RPC, TILES = cfg.CORES, cfg.NPC, cfg.RPC, cfg.TILES
    import ml_dtypes
    iota_np = np.broadcast_to(np.arange(2 * P).astype(ml_dtypes.bfloat16),
                              (P, 2 * P)).copy()
    in_maps = []
    for c in range(CORES):
        sl = slice(c * NPC, (c + 1) * NPC)
        xT = np.zeros((D, RPC), np.float32)
        xT[:, :NPC] = x[sl].T
        d1c = np.zeros(RPC, np.float32)
        d1c[:NPC] = dinv[sl]
        d1 = d1c.reshape(TILES, P).T.copy()
        d2 = (d1 * d1).astype(np.float32)
        im = {
            "xT": xT.astype(ml_dtypes.bfloat16),
            "dinv1": d1,
            "dinv2": d2,
            "iota": iota_np,
            "idx16": per_core[c]["idx16"],
            "dstrel": per_core[c]["dstrel"],
        }
        for i in range(3):
            im[f"W{i + 1}"] = Ws[i]
            im[f"bias{i + 1}"] = np.broadcast_to(
                bss[i], (P, D)).astype(np.float32).copy()
        in_maps.append(im)
    return in_maps


_CACHE = {}


def kernel(x, edge_index, W1, b1, W2, b2, W3, b3):
    cfg = DEFAULT_CFG
    x = np.asarray(x, dtype=np.float32)
    Ws = [np.asarray(w, dtype=np.float32) for w in (W1, W2, W3)]
    bss = [np.asarray(b, dtype=np.float32) for b in (b1, b2, b3)]

    ei = np.asarray(edge_index)
    key = hash(ei[:, ::997].tobytes()) ^ hash(ei.shape)
    if key not in _CACHE:
        meta, per_core = _preprocess(ei, cfg)
        nc = _build_program(meta, cfg)
        _CACHE[key] = (meta, per_core, nc)
    meta, per_core, nc = _CACHE[key]

    in_maps = make_in_maps(x, Ws, bss, meta, per_core, cfg)
    res = run_bass_kernel_spmd(nc, in_maps, core_ids=list(range(cfg.CORES)))
    out = np.empty((cfg.N, D), np.float32)
    for c in range(cfg.CORES):
        out[c * cfg.NPC:(c + 1) * cfg.NPC] = res.results[c]["out"][:cfg.NPC]
    return out


# revision 13
# speedup vs baseline: 1.6232x; 1.0023x over previous
"""3-layer GCN (GCNConv x3 + LeakyReLU, PyG semantics) on 8 Trainium2 cores.

Strategy (source-partitioned, ReduceScatter):
  - Core c owns nodes [c*NPC, (c+1)*NPC) and processes the edges whose SOURCE
    it owns.  Self loops are added as explicit edges, so a whole layer is
        OUT[d] = lrelu(dinv[d] * sum_{e: dst=d} G[src_e] + bias),
    with G = dinv * (H @ W) computed locally per core (no feature exchange).
  - Per layer: phase A computes G for own nodes into two local DRAM gather
    tables (low/high half of own rows, int16-indexable, 256B rows), then
    dma_gather over globally dst-sorted edges + one-hot matmul segment-sum
    into PSUM per global dst tile (784 tiles across all cores), converted to
    bf16 and written into a row-major [100352, 64] partial-sum table.
  - One ReduceScatter(add) delivers each core the full sum for its own
    12544 rows -- an output-sized collective (~57us) instead of AllGathering
    the whole feature table (~284us).
  - Epilogue per own tile: scale by dinv (ACT), add bias (DVE), LeakyReLU
    (ACT), PE transpose, and the next layer's phase-A matmul (bf16).
  - One-hot q matrices (is_equal(iota, drel)) are split across DVE and Pool;
    PSUM->bf16 converts across ACT/DVE, keeping every engine below the DMA
    roofline.  Gathers use one large call per (gather-group, table-half)
    span to amortize the ~1us SWDGE fixed cost per call.

The Bass program is SPMD: one program, per-core data.  Section lengths are
shared across cores (max over cores, padded with dummy edges whose one-hot
column is zero: dstrel = -1).
"""
import sys

sys.path.insert(0, "/opt/trn_rl_repo")

import numpy as np

import concourse.bacc as bacc
import concourse.mybir as mybir
import concourse.tile as tile
from concourse import library_config
from concourse.bass_utils import run_bass_kernel_spmd
from concourse.masks import make_identity

_F32 = mybir.dt.float32
_BF16 = mybir.dt.bfloat16
_I16 = mybir.dt.int16
P = 128
D = 64
NEG_SLOPE = 0.01


class Cfg:
    def __init__(self, n_nodes=100000, cores=8):
        self.N = n_nodes
        self.CORES = cores
        self.NPC = self.N // cores            # nodes owned per core
        self.TILES = (self.NPC + P - 1) // P  # own-node tiles per core (98)
        self.RPC = self.TILES * P             # padded rows per core (12544)
        self.GR = cores * self.RPC            # global padded rows (100352)
        self.GTILES = cores * self.TILES      # global dst tiles (784)
        self.KB = 1                           # source buckets (gather tables)
        assert self.RPC % self.KB == 0
        self.HALF = self.RPC // self.KB       # rows per gather table (12544)
        assert self.HALF <= 32767
        self.GGT = 28                         # dst tiles per gather group
        assert self.GTILES % self.GGT == 0
        self.NGG = self.GTILES // self.GGT    # gather groups (28)
        self.GT = 7                           # dst tiles per write group
        assert self.GGT % self.GT == 0
        self.WPG = self.GGT // self.GT        # write groups per gather group
        self.HTILES = self.TILES // self.KB   # own tiles per table (98)
        assert self.TILES % self.KB == 0
        self.LAYERS = 3
        self.MSGBUFS = 3
        self.QBUFS = 12
        self.PSABUFS = 3
        # engine split knobs
        self.Q_SPLIT = 1000000      # every Q_SPLIT-th q op goes to Pool
        self.CONV_MOD = 1000000     # converts: i%MOD==0 -> DVE, else ACT


DEFAULT_CFG = Cfg()


def _preprocess(edge_index, cfg):
    """Sort/pack edges; build per-core device arrays and shared metadata."""
    src0 = np.asarray(edge_index[0], dtype=np.int64)
    dst0 = np.asarray(edge_index[1], dtype=np.int64)
    N, CORES, NPC, TILES = cfg.N, cfg.CORES, cfg.NPC, cfg.TILES
    NGG, KB, GGT, HALF = cfg.NGG, cfg.KB, cfg.GGT, cfg.HALF

    src = src0
    dst = dst0

    deg = np.bincount(dst, minlength=N).astype(np.float32) + 1.0
    dinv = (1.0 / np.sqrt(deg)).astype(np.float32)

    owner = src // NPC                      # processing core (source owner)
    slocal = src % NPC
    bucket = slocal // HALF                 # gather-table half
    lrow = (slocal % HALF).astype(np.int16)

    downer = dst // NPC
    dloc = dst % NPC
    gtile = downer * TILES + dloc // P      # global dst tile 0..783
    drel_v = (dloc % P).astype(np.float32)
    grp = gtile // GGT
    tl = gtile % GGT

    counts = np.zeros((CORES, NGG, KB, GGT), dtype=np.int64)
    np.add.at(counts, (owner, grp, bucket, tl), 1)
    order = np.lexsort((tl, bucket, grp, owner))
    s_lrow = lrow[order]
    s_drel = drel_v[order]

    sec_len = counts.max(axis=0)            # [NGG, KB, GGT] shared sections

    # layout: gather group -> span (padded to x128) -> tile sections.
    # Each 128-edge block gets one drel column per covered tile PAIR: edges
    # of the pair's second tile store dstrel+128 and the one-hot is built
    # with a single 256-wide is_equal against iota256.
    ginfos = []
    tot_blocks = 0
    tot_cols = 0
    for g in range(NGG):
        gi = {"blk0": tot_blocks, "col0": tot_cols, "spans": {}, "tb": {},
              "bcols": [], "tmm": {}}
        assert KB == 1
        b = 0
        span_len = int(sec_len[g, b].sum())
        kgb = -(-span_len // P)
        gi["spans"][b] = (0, kgb)
        so = 0
        sec_rng = []                    # (t, start, end) in span coords
        for t in range(GGT):
            stb = int(sec_len[g, b, t])
            if stb == 0:
                continue
            gi["tb"][(b, t)] = (0, 0, 0, so)
            sec_rng.append((t, so, so + stb))
            so += stb
        # per block: covered tiles -> pair columns
        gcol = 0
        bcols = []                      # per block: (colbase, t0, ntiles)
        for j in range(kgb):
            lo, hi = j * P, (j + 1) * P
            tl_cov = [t for (t, a, e) in sec_rng if a < hi and e > lo]
            if not tl_cov:
                bcols.append((gcol, -1, 0))
                continue
            t0 = tl_cov[0]
            nt = len(tl_cov)
            assert tl_cov == list(range(t0, t0 + nt))
            bcols.append((gcol, t0, nt))
            gcol += -(-nt // 2)
        gi["bcols"] = bcols
        # per tile: ordered (block, pair m, side) matmul list
        for (t, a, e) in sec_rng:
            lst = []
            for j in range(a // P, (e - 1) // P + 1):
                colbase, t0, nt = bcols[j]
                if t0 < 0:
                    continue
                m = (t - t0) // 2
                side = (t - t0) % 2
                lst.append((j, m, side))
            gi["tmm"][t] = lst
        gi["blocks"] = kgb
        gi["ncols"] = gcol
        tot_blocks += kgb
        tot_cols += gcol
        ginfos.append(gi)
    tot_idx = tot_blocks * P

    # per-core run starts in the sorted edge list ((c, g, b, t)-major order)
    flat = counts.reshape(-1)
    starts = np.zeros(flat.size, dtype=np.int64)
    starts[1:] = np.cumsum(flat)[:-1]
    starts = starts.reshape(CORES, NGG, KB, GGT)

    per_core = []
    for c in range(CORES):
        lidx_flat = np.zeros(tot_idx, dtype=np.int16)
        drel = np.full((P, tot_cols), -1.0, dtype=np.float32)
        for g in range(NGG):
            gi = ginfos[g]
            bcol_base = np.array([bc[0] for bc in gi["bcols"]], dtype=np.int64)
            bcol_t0 = np.array([bc[1] for bc in gi["bcols"]], dtype=np.int64)
            for b, (bo, kgb) in gi["spans"].items():
                span_i0 = (gi["blk0"] + bo) * P
                for t in range(GGT):
                    if (b, t) not in gi["tb"]:
                        continue
                    so = gi["tb"][(b, t)][3]
                    n = int(counts[c, g, b, t])
                    if n == 0:
                        continue
                    s0 = int(starts[c, g, b, t])
                    pos0 = span_i0 + so
                    lidx_flat[pos0:pos0 + n] = s_lrow[s0:s0 + n]
                    q = so + np.arange(n)
                    j = q // P
                    pp = (pos0 + np.arange(n)) % P
                    toff = t - bcol_t0[j]
                    cols = gi["col0"] + bcol_base[j] + toff // 2
                    drel[pp, cols] = s_drel[s0:s0 + n] + 128.0 * (toff % 2)
        idx16 = np.tile(lidx_flat.reshape(tot_idx // 16, 16).T, (8, 1)).copy()
        per_core.append({"idx16": idx16, "dstrel": drel})

    meta = {
        "ginfos": ginfos,
        "tot_idx": tot_idx,
        "tot_cols": tot_cols,
        "dinv": dinv,
    }
    return meta, per_core


def _build_program(meta, cfg):
    ginfos = meta["ginfos"]
    tot_idx = meta["tot_idx"]
    tot_cols = meta["tot_cols"]
    CORES, TILES, RPC = cfg.CORES, cfg.TILES, cfg.RPC
    NGG, KB, GGT, GT = cfg.NGG, cfg.KB, cfg.GGT, cfg.GT
    HALF, HTILES, WPG, GR = cfg.HALF, cfg.HTILES, cfg.WPG, cfg.GR

    kmax_g = max(gi["blocks"] for gi in ginfos)

    nc = bacc.Bacc("TRN2", debug=False)
    nc.num_devices = CORES

    xT_in = nc.dram_tensor("xT", [D, RPC], _BF16, kind="ExternalInput")
    dinv1_in = nc.dram_tensor("dinv1", [P, TILES], _F32, kind="ExternalInput")
    dinv2_in = nc.dram_tensor("dinv2", [P, TILES], _F32, kind="ExternalInput")
    w_in = [nc.dram_tensor(f"W{i + 1}", [D, D], _F32, kind="ExternalInput")
            for i in range(3)]
    bias_in = [nc.dram_tensor(f"bias{i + 1}", [P, D], _F32,
                              kind="ExternalInput") for i in range(3)]
    iota_in = nc.dram_tensor("iota", [P, 2 * P], _BF16,
                            kind="ExternalInput")
    idx_in = nc.dram_tensor("idx16", [P, tot_idx // 16], _I16,
                            kind="ExternalInput")
    drel_in = nc.dram_tensor("dstrel", [P, tot_cols], _F32,
                             kind="ExternalInput")
    out_t = nc.dram_tensor("out", [RPC, D], _F32, kind="ExternalOutput")

    with tile.TileContext(nc) as tc:
        with tc.tile_pool(name="dram", bufs=1, space="DRAM") as dram, \
             tc.tile_pool(name="const", bufs=1) as cst, \
             tc.tile_pool(name="msgp", bufs=cfg.MSGBUFS) as msgp, \
             tc.tile_pool(name="qp", bufs=cfg.QBUFS) as qp, \
             tc.tile_pool(name="gsp", bufs=2) as gsp, \
             tc.tile_pool(name="psp", bufs=3) as psp, \
             tc.tile_pool(name="rsp", bufs=1) as rsp, \
             tc.tile_pool(name="wk", bufs=4) as wk, \
             tc.tile_pool(name="htp", bufs=4) as htpp, \
             tc.tile_pool(name="psa", bufs=cfg.PSABUFS, space="PSUM") as psa, \
             tc.tile_pool(name="psg", bufs=2, space="PSUM") as psg, \
             tc.tile_pool(name="pst", bufs=2, space="PSUM") as pst:

            nc.gpsimd.load_library(library_config.mlp)

            gtabs = [[dram.tile([HALF, 2 * D], _BF16, name=f"gt{L}_{b}")
                      for b in range(KB)] for L in range(cfg.LAYERS)]
            partials = [dram.tile([GR, D], _BF16, name=f"part{L}")
                        for L in range(cfg.LAYERS)]
            rsouts = [dram.tile([RPC, D], _BF16, name=f"rsout{L}")
                      for L in range(cfg.LAYERS)]

            iota = cst.tile([P, 2 * P], _BF16)
            nc.sync.dma_start(iota[:], iota_in[:])
            ident = cst.tile([P, P], _F32)
            make_identity(nc, ident[:])
            dinv1 = cst.tile([P, TILES], _F32)
            nc.sync.dma_start(dinv1[:], dinv1_in[:])
            dinv2 = cst.tile([P, TILES], _F32)
            nc.sync.dma_start(dinv2[:], dinv2_in[:])
            g2b = cst.tile([P, TILES * D], _BF16)
            ws, bs = [], []
            for i in range(3):
                w = cst.tile([D, D], _F32, name=f"w{i}")
                nc.sync.dma_start(w[:], w_in[i][:])
                wb = cst.tile([D, D], _BF16, name=f"wb{i}")
                nc.vector.tensor_copy(wb[:], w[:])
                ws.append(wb)
                bt = cst.tile([P, D], _F32, name=f"b{i}")
                nc.sync.dma_start(bt[:], bias_in[i][:])
                bs.append(bt)
            ht0 = cst.tile([D, RPC], _BF16)        # layer-0 input (x.T)
            nc.sync.dma_start(ht0[:], xT_in[:])

            # zero upper halves of the gather tables (once)
            zstage = cst.tile([P, HTILES * D], _BF16)
            nc.gpsimd.memset(zstage[:], 0.0)
            for L in range(cfg.LAYERS):
                for b in range(KB):
                    dst = gtabs[L][b][:, D:2 * D].rearrange(
                        "(j p) c -> p j c", j=HTILES, p=P)
                    nc.sync.dma_start(dst, zstage[:].rearrange(
                        "p (j c) -> p j c", j=HTILES, c=D))

            idx_sb = cst.tile([P, tot_idx // 16], _I16)
            nc.sync.dma_start(idx_sb[:], idx_in[:])
            drel_sb = cst.tile([P, tot_cols], _F32)
            nc.sync.dma_start(drel_sb[:], drel_in[:])

            hstage = cst.tile([P, TILES * D], _F32)

            qctr = [0]
            cctr = [0]

            def build_q(qt, col, width):
                # wide (256) one-hots go to DVE (2-elem/cycle); narrow ones
                # alternate DVE/Pool per Q_SPLIT
                if width > P:
                    eng = nc.vector
                else:
                    eng = (nc.gpsimd
                           if (qctr[0] % cfg.Q_SPLIT == cfg.Q_SPLIT - 1)
                           else nc.vector)
                    qctr[0] += 1
                eng.tensor_scalar(
                    out=qt[:, :width], in0=iota[:, :width],
                    scalar1=drel_sb[:, col:col + 1], scalar2=None,
                    op0=mybir.AluOpType.is_equal)

            def convert(dst_ap, src_ap):
                m = cctr[0] % cfg.CONV_MOD
                cctr[0] += 1
                if m == 0:
                    nc.vector.tensor_copy(dst_ap, src_ap)
                else:
                    nc.scalar.copy(dst_ap, src_ap)

            def emit_gathers(L, g):
                gi = ginfos[g]
                msg = msgp.tile([P, gi["blocks"], 2 * D], _BF16,
                                tag="msg", name=f"msg{L}_{g}",
                                padded_shape=[P, kmax_g, 2 * D])
                for b, (bo, kgb) in gi["spans"].items():
                    i0 = (gi["blk0"] + bo) * P
                    nidx = kgb * P
                    nc.gpsimd.dma_gather(
                        msg[:, bo:bo + kgb, :],
                        gtabs[L][b][:],
                        idx_sb[:, i0 // 16:(i0 + nidx) // 16],
                        nidx, nidx, 2 * D,
                        single_packet=False)
                return msg

            def emit_process(L, g, msg):
                gi = ginfos[g]
                bcols = gi["bcols"]
                qtiles = {}                      # (j, m) -> q tile

                def get_q(j, m):
                    key = (j, m)
                    if key not in qtiles:
                        colbase, t0, nt = bcols[j]
                        width = 2 * P if (nt - 2 * m) >= 2 else P
                        qt = qp.tile([P, 2 * P], _BF16, tag="q",
                                     name=f"q{L}_{g}_{j}_{m}")
                        build_q(qt, gi["col0"] + colbase + m, width)
                        qtiles[key] = qt
                    return qtiles[key]

                for wg in range(WPG):
                    pstage = psp.tile([P, GT * D], _BF16, tag="ps",
                                      name=f"ps{L}_{g}_{wg}")
                    pa = psa.tile([P, GT * D], _F32, tag="pa",
                                  name=f"pa{L}_{g}_{wg}")
                    for tw in range(GT):
                        t = wg * GT + tw
                        tmm = gi["tmm"].get(t, [])
                        nmm = len(tmm)
                        assert nmm > 0
                        for done, (j, m, side) in enumerate(tmm):
                            qt = get_q(j, m)
                            nc.tensor.matmul(
                                pa[:, tw * D:(tw + 1) * D],
                                lhsT=qt[:, side * P:(side + 1) * P],
                                rhs=msg[:, j, :D],
                                start=(done == 0),
                                stop=(done == nmm - 1))
                    convert(pstage[:], pa[:])
                    r0 = (g * GGT + wg * GT) * P
                    dst = partials[L][r0:r0 + GT * P, :].rearrange(
                        "(j p) c -> p j c", j=GT, p=P)
                    nc.sync.dma_start(dst, pstage[:].rearrange(
                        "p (j c) -> p j c", j=GT, c=D))

            def emit_phase_a(L):
                """Epilogue of layer L-1 (if L>0) fused with phase A of L."""
                rsall = None
                if L > 0:
                    rsall = rsp.tile([P, TILES * D], _BF16, tag="rs",
                                     name=f"rsall{L}")
                    src = rsouts[L - 1][:].rearrange(
                        "(j p) c -> p j c", j=TILES, p=P)
                    nc.sync.dma_start(rsall[:].rearrange(
                        "p (j c) -> p j c", j=TILES, c=D), src)
                for b in range(KB):
                    gstage = gsp.tile([P, HTILES * D], _BF16, tag="gs",
                                      name=f"gs{L}_{b}")
                    for th in range(HTILES):
                        t = b * HTILES + th
                        if L == 0:
                            lhs = ht0[:, t * P:(t + 1) * P]
                        else:
                            v = wk.tile([P, D], _BF16, tag="v",
                                        name=f"v{L}_{t}")
                            nc.scalar.mul(v[:], rsall[:, t * D:(t + 1) * D],
                                          dinv1[:, t:t + 1])
                            nc.vector.tensor_tensor(
                                out=v[:], in0=v[:],
                                in1=g2b[:, t * D:(t + 1) * D],
                                op=mybir.AluOpType.add)
                            h = wk.tile([P, D], _F32, tag="h",
                                        name=f"h{L}_{t}")
                            nc.scalar.activation(
                                h[:], v[:],
                                mybir.ActivationFunctionType.Lrelu,
                                bias=0.0, scale=1.0, alpha=NEG_SLOPE)
                            pt = pst.tile([D, P], _F32, tag="pt",
                                          name=f"pt{L}_{t}")
                            nc.tensor.transpose(pt[:], h[:], ident[:])
                            htp = htpp.tile([D, P], _BF16, tag="ht",
                                            name=f"htp{L}_{t}")
                            nc.vector.tensor_copy(htp[:], pt[:])
                            lhs = htp[:]
                        pg = psg.tile([P, D], _F32, tag="pg",
                                      name=f"pg{L}_{t}")
                        nc.tensor.matmul(pg[:], lhsT=lhs, rhs=ws[L][:],
                                         start=True, stop=True)
                        nc.vector.tensor_scalar_mul(
                            gstage[:, th * D:(th + 1) * D], pg[:],
                            dinv1[:, t:t + 1])
                        g2 = wk.tile([P, D], _F32, tag="g2",
                                     name=f"g2_{L}_{t}")
                        nc.vector.tensor_scalar_mul(g2[:], pg[:],
                                                    dinv2[:, t:t + 1])
                        nc.vector.tensor_tensor(
                            out=g2b[:, t * D:(t + 1) * D], in0=g2[:],
                            in1=bs[L][:], op=mybir.AluOpType.add)
                    dst = gtabs[L][b][:, :D].rearrange(
                        "(j p) c -> p j c", j=HTILES, p=P)
                    nc.sync.dma_start(dst, gstage[:].rearrange(
                        "p (j c) -> p j c", j=HTILES, c=D))

            for L in range(cfg.LAYERS):
                emit_phase_a(L)
                msgs = {0: emit_gathers(L, 0)}
                if NGG > 1:
                    msgs[1] = emit_gathers(L, 1)
                for g in range(NGG):
                    if g + 2 < NGG:
                        msgs[g + 2] = emit_gathers(L, g + 2)
                    emit_process(L, g, msgs.pop(g))
                nc.gpsimd.collective_compute(
                    "ReduceScatter",
                    mybir.AluOpType.add,
                    replica_groups=[list(range(CORES))],
                    ins=[partials[L][:]],
                    outs=[rsouts[L][:]],
                )

            # ---------- final epilogue -> output ----------
            L = cfg.LAYERS
            rsall = rsp.tile([P, TILES * D], _BF16, tag="rs", name="rsall_f")
            src = rsouts[L - 1][:].rearrange("(j p) c -> p j c", j=TILES, p=P)
            nc.sync.dma_start(rsall[:].rearrange(
                "p (j c) -> p j c", j=TILES, c=D), src)
            for t in range(TILES):
                v = wk.tile([P, D], _BF16, tag="v", name=f"vf_{t}")
                nc.scalar.mul(v[:], rsall[:, t * D:(t + 1) * D],
                              dinv1[:, t:t + 1])
                nc.vector.tensor_tensor(
                    out=v[:], in0=v[:],
                    in1=g2b[:, t * D:(t + 1) * D],
                    op=mybir.AluOpType.add)
                nc.scalar.activation(
                    hstage[:, t * D:(t + 1) * D], v[:],
                    mybir.ActivationFunctionType.Lrelu,
                    bias=0.0, scale=1.0, alpha=NEG_SLOPE)
            dst = out_t[:].rearrange("(j p) c -> p j c", j=TILES, p=P)
            nc.sync.dma_start(dst, hstage[:].rearrange(
                "p (j c) -> p j c", j=TILES, c=D))

    nc.compile()
    return nc


def make_in_maps(x, Ws, bss, meta, per_core, cfg):
    dinv = meta["dinv"]
    CORES, NPC, RPC, TILES = cfg.CORES, cfg.NPC, cfg.RPC, cfg.TILES
    import ml_dtypes
    iota_np = np.broadcast_to(np.arange(2 * P).astype(ml_dtypes.bfloat16),
                              (P, 2 * P)).copy()
    in_maps = []
    for c in range(CORES):
        sl = slice(c * NPC, (c + 1) * NPC)
        xT = np.zeros((D, RPC), np.float32)
        xT[:, :NPC] = x[sl].T
        d1c = np.zeros(RPC, np.float32)
        d1c[:NPC] = dinv[sl]
        d1 = d1c.reshape(TILES, P).T.copy()
        d2 = (d1 * d1).astype(np.float32)
        im = {
            "xT": xT.astype(ml_dtypes.bfloat16),
            "dinv1": d1,
            "dinv2": d2,
            "iota": iota_np,
            "idx16": per_core[c]["idx16"],
            "dstrel": per_core[c]["dstrel"],
        }
        for i in range(3):
            im[f"W{i + 1}"] = Ws[i]
            im[f"bias{i + 1}"] = np.broadcast_to(
                bss[i], (P, D)).astype(np.float32).copy()
        in_maps.append(im)
    return in_maps


_CACHE = {}


def kernel(x, edge_index, W1, b1, W2, b2, W3, b3):
    cfg = DEFAULT_CFG
    x = np.asarray(x, dtype=np.float32)
    Ws = [np.asarray(w, dtype=np.float32) for w in (W1, W2, W3)]
    bss = [np.asarray(b, dtype=np.float32) for b in (b1, b2, b3)]

    ei = np.asarray(edge_index)
    key = hash(ei[:, ::997].tobytes()) ^ hash(ei.shape)
    if key not in _CACHE:
        meta, per_core = _preprocess(ei, cfg)
        nc = _build_program(meta, cfg)
        _CACHE[key] = (meta, per_core, nc)
    meta, per_core, nc = _CACHE[key]

    in_maps = make_in_maps(x, Ws, bss, meta, per_core, cfg)
    res = run_bass_kernel_spmd(nc, in_maps, core_ids=list(range(cfg.CORES)))
    out = np.empty((cfg.N, D), np.float32)
    for c in range(cfg.CORES):
        out[c * cfg.NPC:(c + 1) * cfg.NPC] = res.results[c]["out"][:cfg.NPC]
    return out
